# revision 25
# baseline (speedup 1.0000x reference)
"""CausalGraphBuilder Trainium2 kernel.

Full inputs -> shard batch (128) over 8 NeuronCores (16 each) -> Bass kernel
(encoder MLP + LayerNorm + N^2 pair-MLP edge/strength heads) -> gather.

Self-contained: hardcodes B,S,D,N = 128,1024,512,64 and the sharding.
"""

import numpy as np

B, S, D, N = 128, 1024, 512, 64
DM = D // 4          # 128 node-feature dim after encoder
NCORES = 8
BSH = B // NCORES    # 16 batch per core
TOK = BSH * N        # 1024 tokens per core

# jnp.linspace(0, S-1, N).astype(int32) as evaluated by the reference in this
# environment (device cast rounds); recomputed at runtime when jax is available.
_FALLBACK_IDX = [0, 16, 32, 49, 65, 81, 97, 114, 130, 146, 162, 179, 195, 211,
                 227, 244, 260, 276, 292, 309, 325, 341, 357, 373, 390, 406,
                 422, 438, 455, 471, 487, 503, 520, 536, 552, 568, 585, 601,
                 617, 633, 650, 666, 682, 698, 714, 731, 747, 763, 779, 796,
                 812, 828, 844, 861, 877, 893, 909, 926, 942, 958, 974, 991,
                 1007, 1023]


def _node_indices():
    try:
        import jax.numpy as jnp
        idx = np.asarray(jnp.linspace(0.0, float(S - 1), N).astype(jnp.int32))
        if idx.shape == (N,):
            return idx.astype(np.int64)
    except Exception:
        pass
    return np.array(_FALLBACK_IDX, dtype=np.int64)


# ---------------------------------------------------------------------------
# device program
# ---------------------------------------------------------------------------

_PROGRAM_CACHE = {}
LAST_RESULTS = None  # BassKernelResults of the most recent run (for test.py)

# engine assignment knobs (tuned against the profile)
PE_E_TILES = ()             # E-groups built on PE (identity-matmul 2-pass)
PE_S_TILES = (1, 3)     # S-groups built on PE
EVAC_DVE_E = ()               # PE-built E tiles evacuated by DVE instead of ACT
EVAC_DVE_S = ()
GP_RELU = False               # DVE-built tiles: relu on GPSIMD (slow ucode!)
E2RELU_DVE = ()               # e2 cp-chunks (0..3) whose relu-evac runs on DVE



# packed-constant blob layouts (columns)
_W16_SECTIONS = [
    ("w1_0", 256), ("w1_1", 256), ("w1_2", 256), ("w1_3", 256),
    ("w2_0", 128), ("w2_1", 128),
    ("we1a", 64), ("we1b", 64), ("ws1a", 32), ("ws1b", 32),
    ("bd2we2", 64), ("bd4we3", 4), ("bd4ws2", 4),
]
W16_OFF = {}
_o = 0
for _k, _w in _W16_SECTIONS:
    W16_OFF[_k] = (_o, _w)
    _o += _w
W16_COLS = _o
_W32_SECTIONS = [("b1_2", 2), ("b2c", 1), ("gammap", 1), ("betap", 1),
                 ("be1_2", 1), ("bs1_4", 1), ("be2_4", 1), ("be3b", 1),
                 ("bs2b", 1), ("sigb", 1), ("sigs", 1), ("beA", 1),
                 ("beB", 1), ("bsA", 1), ("bsB", 1)]
W32_OFF = {}
_o = 0
for _k, _w in _W32_SECTIONS:
    W32_OFF[_k] = _o
    _o += _w
W32_COLS = _o

def _patched_tile_context(nc):
    """TileContext whose tail drain never carries >1 sync wait (this walrus
    build rejects multi-wait CTRL instructions)."""
    import concourse.mybir as mybir
    import concourse.tile as tile
    from concourse.vector_clock import ScopedClock

    class TileContextP(tile.TileContext):
        def _drain_and_barrier(self, tick_clock, wait_clock):
            drain_inst = self.nc.sync.drain()
            wait_clock.add_sem_waits(
                drain_inst.ins, ScopedClock({None: tick_clock.global_clock})
            )
            si = drain_inst.ins.sync_info
            if si is not None and si.on_wait is not None and len(si.on_wait) > 1:
                waits = list(si.on_wait)
                si.on_wait = waits[:1]
                for w in waits[1:]:
                    extra = self.nc.sync.drain()
                    extra.ins.sync_info = mybir.SyncInfo(on_wait=[w], on_update=[])
            self.nc.all_engine_barrier()
            popped = self.nc._tile_sem_poison_stack.pop()
            assert popped is self._sem_poison
            self.nc.clear_and_free_semaphores(list(self.sems.allocated().values()))

    return TileContextP(nc)


def _split_multiwait(nc):
    """This walrus build accepts at most ONE sync wait per instruction; hoist
    extra waits into single-wait NoOps on the same engine just before."""
    import concourse.mybir as mybir

    n_split = 0
    for f in nc.m.functions:
        for bb in f.blocks:
            insts = list(bb.instructions)
            out = []
            for ins in insts:
                si = ins.sync_info
                if si is not None and si.on_wait is not None and len(si.on_wait) > 1:
                    waits = list(si.on_wait)
                    for w in waits[:-1]:
                        nop = mybir.InstNoOp(
                            name=f"{ins.name}-w{n_split}",
                            engine=ins.engine,
                            bass_nofuse=True,
                            sync_info=mybir.SyncInfo(on_wait=[w], on_update=[]),
                        )
                        out.append(nop)
                        n_split += 1
                    si.on_wait = waits[-1:]
                out.append(ins)
            if n_split:
                bb.instructions = out
    return n_split


def _build_program():
    import concourse.bass as bass
    import concourse.mybir as mybir
    from concourse.masks import make_identity

    f16 = mybir.dt.float16
    f32 = mybir.dt.float32
    AF = mybir.ActivationFunctionType
    OP = mybir.AluOpType

    nc = bass.Bass()

    # ---- DRAM I/O ----
    xt_d = nc.dram_tensor("xt", [D, TOK], f16, kind="ExternalInput")
    wb16_d = nc.dram_tensor("wb16", [128, W16_COLS], f16, kind="ExternalInput")
    wb32_d = nc.dram_tensor("wb32", [128, W32_COLS], f32, kind="ExternalInput")

    nf_d = nc.dram_tensor("nf_raw", [TOK, DM], f32, kind="ExternalOutput")
    pairs_d = nc.dram_tensor("pairs", [2, 128, N * N], f16, kind="ExternalOutput")

    with _patched_tile_context(nc) as tc:
        from contextlib import ExitStack

        with ExitStack() as ctx:
            consts = ctx.enter_context(tc.tile_pool(name="consts", bufs=1))
            persist = ctx.enter_context(tc.tile_pool(name="persist", bufs=1))

            # ---- load constants ----
            def load(pool, name, dram, shape, dtype):
                t = pool.tile(shape, dtype, tag=name, name=name)
                nc.sync.dma_start(out=t[:], in_=dram[:])
                return t

            wb32 = load(consts, "wb32", wb32_d, [128, W32_COLS], f32)
            wb16 = load(consts, "wb16", wb16_d, [128, W16_COLS], f16)
            dummy_sig = consts.tile([128, 1], f32, tag="dsig", name="dummy_sig")
            nc.scalar.activation(out=dummy_sig[:], in_=wb32[:, 0:1],
                                 func=AF.Sigmoid)
            xt_sb4 = consts.tile([128, 4, TOK], f16, tag="xt", name="xt_sb4")
            for c in range(4):
                eng = nc.gpsimd if c % 2 == 0 else nc.sync
                eng.dma_start(
                    out=xt_sb4[:, c, :], in_=xt_d[c * 128:(c + 1) * 128, :])
            xt_sb = [xt_sb4[:, c, :] for c in range(4)]

            def c16(key):
                o, w = W16_OFF[key]
                return wb16[:, o:o + w]

            def c32(key):
                o = W32_OFF[key]
                return wb32[:, o:o + 1]

            w1_sb = [c16(f"w1_{c}") for c in range(4)]
            w2_sb = [c16(f"w2_{c}") for c in range(2)]
            we1a_sb = c16("we1a"); we1b_sb = c16("we1b")
            ws1a_sb = c16("ws1a"); ws1b_sb = c16("ws1b")
            bd2we2_sb = c16("bd2we2")
            bd4we3_sb = c16("bd4we3"); bd4ws2_sb = c16("bd4ws2")
            b1_sb = wb32[:, W32_OFF["b1_2"]:W32_OFF["b1_2"] + 2]
            b2_sb = c32("b2c"); gp_sb = c32("gammap"); bp_sb = c32("betap")
            be1_sb = c32("be1_2"); bs1_sb = c32("bs1_4"); be2_sb = c32("be2_4")
            be3_sb = c32("be3b"); bs2_sb = c32("bs2b")
            sigb_sb = c32("sigb"); sigs_sb = c32("sigs")
            beA_sb = c32("beA"); beB_sb = c32("beB")
            bsA_sb = c32("bsA"); bsB_sb = c32("bsB")

            eps_sb = consts.tile([128, 1], f32, tag="eps")
            nc.vector.memset(eps_sb[:], 1e-5)

            idf32 = consts.tile([128, 128], f32, tag="idf32")
            make_identity(nc, idf32[:])
            idf16 = consts.tile([128, 128], f16, tag="idf16")
            make_identity(nc, idf16[:])

            h1t = [persist.tile([128, TOK], f16, tag=f"h1t{m}", name=f"h1t{m}") for m in range(2)]
            ht = persist.tile([128, TOK], f32, tag="ht")
            nfraw = persist.tile([128, 8, 128], f32, tag="nfraw")
            nft = persist.tile([128, TOK], f16, tag="nft")
            ab_all = persist.tile([128, 8, 192], f16, tag="ab_all")
            ab = [ab_all[:, g, :] for g in range(8)]
            apbp_all = persist.tile([128, 4, 192], f16, tag="apbp_all")
            apbp = [apbp_all[:, gr, :] for gr in range(4)]
            e2r = [persist.tile([128, N * N], f16, tag=f"e2r{u}", name=f"e2r{u}") for u in range(4)]
            s1r = [persist.tile([128, N * N], f16, tag=f"s1r{gr}", name=f"s1r{gr}") for gr in range(4)]

            # ================= encoder =================
            with tc.tile_pool(name="ps_enc", bufs=2, space="PSUM") as ps_enc:
                for m in range(2):
                    for n2 in range(2):
                        ps = ps_enc.tile([128, 512], f32, tag="enc", name="ps_enc_t")
                        for c in range(4):
                            nc.tensor.matmul(
                                ps[:],
                                lhsT=w1_sb[c][:, m * 128:(m + 1) * 128],
                                rhs=xt_sb[c][:, n2 * 512:(n2 + 1) * 512],
                                start=(c == 0), stop=(c == 3),
                            )
                        nc.scalar.activation(
                            out=h1t[m][:, n2 * 512:(n2 + 1) * 512], in_=ps[:],
                            func=AF.Relu, bias=b1_sb[:, m:m + 1], scale=1.0,
                        )
                for n2 in range(2):
                    ps = ps_enc.tile([128, 512], f32, tag="enc", name="ps_enc_t")
                    for c2 in range(2):
                        nc.tensor.matmul(
                            ps[:], lhsT=w2_sb[c2][:],
                            rhs=h1t[c2][:, n2 * 512:(n2 + 1) * 512],
                            start=(c2 == 0), stop=(c2 == 1),
                        )
                    nc.scalar.activation(
                        out=ht[:, n2 * 512:(n2 + 1) * 512], in_=ps[:],
                        func=AF.Identity, bias=b2_sb[:, 0:1], scale=1.0,
                    )

            # ================= layernorm (token-major) =================
            # sqrt-free: rstd = Newton-rsqrt from 1/x seed (keeps a single
            # ACT table set resident for the whole kernel)
            mu_all = persist.tile([128, 8], f32, tag="mu_all")
            rstd_all = persist.tile([128, 8], f32, tag="rstd_all")
            ve_all = persist.tile([128, 8], f32, tag="ve_all")
            nmr_all = persist.tile([128, 8], f32, tag="nmr_all")
            with tc.tile_pool(name="ps_tr", bufs=1, space="PSUM") as ps_tr, \
                 tc.tile_pool(name="ln_tmp", bufs=4) as ln_tmp:
                pstok = ps_tr.tile([128, 1024], f32, tag="htok", name="pstok")
                for t in range(8):
                    nc.tensor.transpose(pstok[:, t * 128:(t + 1) * 128],
                                        ht[:, t * 128:(t + 1) * 128], idf32[:])
                st6 = ln_tmp.tile([128, 8, 6], f32, tag="st6", name="st6")
                p3 = pstok[:].rearrange("p (t f) -> p t f", t=8)
                for t in range(8):
                    nc.vector.bn_stats(out=st6[:, t, :], in_=p3[:, t, :])
                mv3 = ln_tmp.tile([128, 8, 2], f32, tag="mv3", name="mv3")
                for t in range(8):
                    nc.vector.bn_aggr(out=mv3[:, t, :], in_=st6[:, t, :])
                nc.vector.tensor_copy(mu_all[:], mv3[:, :, 0])
                # rstd = rsqrt(ve + eps), Newton x3 from linear-in-1/x seed
                nc.vector.tensor_scalar_add(out=ve_all[:], in0=mv3[:, :, 1],
                                            scalar1=1e-5)
                rr = ln_tmp.tile([128, 8], f32, tag="rr", name="rr")
                nc.vector.reciprocal(out=rr[:], in_=ve_all[:])
                nc.vector.tensor_scalar(out=rstd_all[:], in0=rr[:],
                                        scalar1=0.345, scalar2=0.53,
                                        op0=OP.mult, op1=OP.add)
                t2 = ln_tmp.tile([128, 8], f32, tag="t2", name="t2")
                for _ in range(3):
                    nc.vector.tensor_mul(t2[:], rstd_all[:], rstd_all[:])
                    nc.vector.tensor_mul(t2[:], t2[:], ve_all[:])
                    nc.vector.tensor_scalar(out=t2[:], in0=t2[:],
                                            scalar1=-0.5, scalar2=1.5,
                                            op0=OP.mult, op1=OP.add)
                    nc.vector.tensor_mul(rstd_all[:], rstd_all[:], t2[:])
                nc.vector.tensor_mul(nmr_all[:], mu_all[:], rstd_all[:])
                nc.vector.tensor_scalar_mul(out=nmr_all[:], in0=nmr_all[:],
                                            scalar1=-1.0)
                for t in range(8):
                    nc.scalar.activation(
                        out=nfraw[:, t, :], in_=pstok[:, t * 128:(t + 1) * 128],
                        func=AF.Identity, bias=nmr_all[:, t:t + 1],
                        scale=rstd_all[:, t:t + 1])
                nc.sync.dma_start(
                    out=nf_d[:].rearrange("(t p) f -> p t f", t=8, p=128),
                    in_=nfraw[:])

                # nfT (feature-major) with gamma/beta applied per-partition
                psn = ps_tr.tile([128, 1024], f32, tag="nft_ps", name="psn")
                for t in range(8):
                    nc.tensor.transpose(psn[:, t * 128:(t + 1) * 128],
                                        nfraw[:, t, :], idf32[:])
                nc.scalar.activation(out=nft[:], in_=psn[:], func=AF.Copy)

            # ================= pair projections =================
            # Fat matmuls with strided token-gather APs: for each partition
            # parity bb, one matmul gathers the i-side (A) columns of all its
            # groups; the j-side (B) is emitted with duplicated columns for
            # the 2x-mode pair build. Bias be1/bs1 is added as bias/2 on both
            # the A and B evacuations.
            with tc.tile_pool(name="ps_proj", bufs=1, space="PSUM") as ps_proj:
                psE = ps_proj.tile([128, 1536], f32, tag="proj_e",
                                   name="psE")
                nftv = nft[:]
                for bb in range(2):
                    # A: [64f, 8 groups x 64 cols]; tokens of b = 2g + bb
                    rhs_a = bass.AP(
                        tensor=nftv.tensor, offset=nftv.offset + bb * N,
                        ap=[list(nftv.ap[0]), [2 * N, 8], [1, N]])
                    nc.tensor.matmul(
                        psE[64 * bb:64 * bb + 64, 0:512], lhsT=we1a_sb[:],
                        rhs=rhs_a, start=True, stop=True,
                        tile_position=(0, 64 * bb))
                    # B dup: [64f, 8 groups x 128 cols] (two 512-col mms)
                    for hf in range(2):
                        rhs_b = bass.AP(
                            tensor=nftv.tensor,
                            offset=nftv.offset + bb * N + hf * 4 * 2 * N,
                            ap=[list(nftv.ap[0]), [2 * N, 4], [1, N], [0, 2]])
                        nc.tensor.matmul(
                            psE[64 * bb:64 * bb + 64,
                                512 + hf * 512:512 + (hf + 1) * 512],
                            lhsT=we1b_sb[:], rhs=rhs_b, start=True, stop=True,
                            tile_position=(0, 64 * bb))
                nc.scalar.activation(
                    out=ab_all[:, :, 0:64],
                    in_=psE[:, 0:512].rearrange("p (g n) -> p g n", g=8),
                    func=AF.Identity, bias=beA_sb[:, 0:1], scale=1.0)
                nc.scalar.activation(
                    out=ab_all[:, :, 64:192],
                    in_=psE[:, 512:1536].rearrange("p (g n) -> p g n", g=8),
                    func=AF.Identity, bias=beB_sb[:, 0:1], scale=1.0)

                psS = ps_proj.tile([128, 1024], f32, tag="proj_s", name="psS")
                for bb in range(4):
                    rhs_a = bass.AP(
                        tensor=nftv.tensor, offset=nftv.offset + bb * N,
                        ap=[list(nftv.ap[0]), [4 * N, 4], [1, N]])
                    nc.tensor.matmul(
                        psS[32 * bb:32 * bb + 32, 0:256], lhsT=ws1a_sb[:],
                        rhs=rhs_a, start=True, stop=True,
                        tile_position=(0, 32 * bb))
                    rhs_b = bass.AP(
                        tensor=nftv.tensor, offset=nftv.offset + bb * N,
                        ap=[list(nftv.ap[0]), [4 * N, 4], [1, N], [0, 2]])
                    nc.tensor.matmul(
                        psS[32 * bb:32 * bb + 32, 512:1024], lhsT=ws1b_sb[:],
                        rhs=rhs_b, start=True, stop=True,
                        tile_position=(0, 32 * bb))
                nc.scalar.activation(
                    out=apbp_all[:, :, 0:64],
                    in_=psS[:, 0:256].rearrange("p (g n) -> p g n", g=4),
                    func=AF.Identity, bias=bsA_sb[:, 0:1], scale=1.0)
                nc.scalar.activation(
                    out=apbp_all[:, :, 64:192],
                    in_=psS[:, 512:1024].rearrange("p (g n) -> p g n", g=4),
                    func=AF.Identity, bias=bsB_sb[:, 0:1], scale=1.0)

            # ================= N^2 builds + edge layer2 =================
            # E-path: per u (2 edge groups): DVE builds -> PE e2 -> ACT evac,
            # with finals-half0 matmuls interleaved (PSUM: e2 4 banks +
            # finals-h0 4 banks). S-path runs at the tail, overlapping
            # E-finals/sigmoid.
            relu_act_counter = [0]

            def build_dve(out3, src):
                out4 = out3.rearrange("p j (i2 ii) -> p j i2 ii", ii=2)
                a4 = src[:, None, 0:64].broadcast_to((128, N, 64)) \
                    .rearrange("p j (i2 ii) -> p j i2 ii", ii=2)
                b4 = src[:, 64:192].rearrange("p (j ii) -> p j ii", ii=2)[
                    :, :, None, :].broadcast_to((128, N, 32, 2))
                k = relu_act_counter[0]
                relu_act_counter[0] += 1
                flat = out3.rearrange("p i j -> p (i j)")
                for h in range(2):
                    nc.vector.tensor_add(out4[:, 32 * h:32 * h + 32],
                                         b4[:, 32 * h:32 * h + 32],
                                         a4[:, 32 * h:32 * h + 32])
                    hf = flat[:, h * 2048:(h + 1) * 2048]
                    if k % 2 == 1:    # alternate relu between DVE and ACT
                        nc.scalar.activation(out=hf, in_=hf, func=AF.Relu)
                    else:
                        nc.vector.tensor_scalar_max(out=hf, in0=hf,
                                                    scalar1=0.0)

            adj_t = [None, None]
            str_t = [None, None]
            with tc.tile_pool(name="e1pool", bufs=4) as e1pool, \
                 tc.tile_pool(name="ps_e2", bufs=2, space="PSUM") as ps_e2, \
                 tc.tile_pool(name="ps_finE0", bufs=1, space="PSUM") as ps_finE0, \
                 tc.tile_pool(name="outstage", bufs=1) as outstage:
                finE0 = ps_finE0.tile([128, 2048], f32, tag="fe0", name="finE0")
                for u in range(4):
                    e1t = []
                    for g in (2 * u, 2 * u + 1):
                        e1 = e1pool.tile([128, N, N], f16, tag="e1",
                                         name=f"e1_{g}")
                        build_dve(e1[:], ab[g])
                        e1t.append(e1[:].rearrange("p i j -> p (i j)"))
                    for cp in range(4):
                        ps = ps_e2.tile([128, 1024], f32, tag="e2",
                                        name="ps_e2_t")
                        for hf in range(2):
                            c0 = cp * 1024 + hf * 512
                            nc.tensor.matmul(
                                ps[0:64, hf * 512:(hf + 1) * 512],
                                lhsT=bd2we2_sb[:], rhs=e1t[0][:, c0:c0 + 512],
                                start=True, stop=True, tile_position=(0, 0),
                            )
                            nc.tensor.matmul(
                                ps[64:128, hf * 512:(hf + 1) * 512],
                                lhsT=bd2we2_sb[:], rhs=e1t[1][:, c0:c0 + 512],
                                start=True, stop=True, tile_position=(0, 64),
                            )
                        nc.scalar.activation(
                            out=e2r[u][:, cp * 1024:(cp + 1) * 1024], in_=ps[:],
                            func=AF.Relu, bias=be2_sb[:, 0:1], scale=1.0,
                        )
                    for ch in range(4):     # finals half 0 for this u
                        nc.tensor.matmul(
                            finE0[32 * u:32 * u + 4, ch * 512:(ch + 1) * 512],
                            lhsT=bd4we3_sb[:], rhs=e2r[u][:, ch * 512:
                                                          (ch + 1) * 512],
                            start=True, stop=True, tile_position=(0, 32 * u),
                        )
                    build_dve(s1r[u][:].rearrange("p (i j) -> p i j", i=N),
                              apbp[u])
                adj_t[0] = outstage.tile([128, 2048], f16, tag="adj0",
                                         name="adj_t0")
                nc.scalar.activation(out=adj_t[0][:], in_=finE0[:],
                                     func=AF.Sigmoid, bias=be3_sb[:, 0:1],
                                     scale=1.0)
                nc.sync.dma_start(out=pairs_d[0, :, 0:1024],
                                  in_=adj_t[0][:, 0:1024])
                nc.gpsimd.dma_start(out=pairs_d[0, :, 1024:2048],
                                    in_=adj_t[0][:, 1024:2048])

            # tail: S finals h0 -> E finals h1 -> S finals h1
            with tc.tile_pool(name="ps_fin2", bufs=2, space="PSUM") as ps_fin2, \
                 tc.tile_pool(name="outstage2", bufs=2) as outstage2:
                def head_mms(ps, lhsT, rhs_tiles, c0):
                    for ch in range(4):
                        for k in range(4):
                            nc.tensor.matmul(
                                ps[32 * k:32 * k + 4, ch * 512:(ch + 1) * 512],
                                lhsT=lhsT,
                                rhs=rhs_tiles[k][:, c0 + ch * 512:
                                                 c0 + (ch + 1) * 512],
                                start=True, stop=True,
                                tile_position=(0, 32 * k),
                            )

                s1aps = [s1r[gr][:] for gr in range(4)]
                e2aps = [e2r[u][:] for u in range(4)]
                ps_s0 = ps_fin2.tile([128, 2048], f32, tag="fin",
                                     name="ps_s0")
                head_mms(ps_s0, bd4ws2_sb[:], s1aps, 0)
                st0 = outstage2.tile([128, 2048], f16, tag="o2", name="st0")
                nc.scalar.activation(out=st0[:], in_=ps_s0[:], func=AF.Tanh,
                                     bias=bs2_sb[:, 0:1], scale=1.0)
                nc.sync.dma_start(out=pairs_d[1, :, 0:1024],
                                  in_=st0[:, 0:1024])
                nc.gpsimd.dma_start(out=pairs_d[1, :, 1024:2048],
                                    in_=st0[:, 1024:2048])

                ps_e1 = ps_fin2.tile([128, 2048], f32, tag="fin",
                                     name="ps_e1f")
                head_mms(ps_e1, bd4we3_sb[:], e2aps, 2048)
                at1 = outstage2.tile([128, 2048], f16, tag="o2", name="at1")
                nc.scalar.activation(out=at1[:], in_=ps_e1[:], func=AF.Sigmoid,
                                     bias=be3_sb[:, 0:1], scale=1.0)
                nc.sync.dma_start(out=pairs_d[0, :, 2048:3072],
                                  in_=at1[:, 0:1024])
                nc.gpsimd.dma_start(out=pairs_d[0, :, 3072:4096],
                                    in_=at1[:, 1024:2048])

                ps_s1 = ps_fin2.tile([128, 2048], f32, tag="fin",
                                     name="ps_s1f")
                head_mms(ps_s1, bd4ws2_sb[:], s1aps, 2048)
                st1 = outstage2.tile([128, 2048], f16, tag="o2", name="st1")
                nc.scalar.activation(out=st1[:], in_=ps_s1[:], func=AF.Tanh,
                                     bias=bs2_sb[:, 0:1], scale=1.0)
                nc.sync.dma_start(out=pairs_d[1, :, 2048:3072],
                                  in_=st1[:, 0:1024])
                nc.gpsimd.dma_start(out=pairs_d[1, :, 3072:4096],
                                    in_=st1[:, 1024:2048])

    _split_multiwait(nc)
    return nc


def _get_program():
    if "nc" not in _PROGRAM_CACHE:
        _PROGRAM_CACHE["nc"] = _build_program()
    return _PROGRAM_CACHE["nc"]


# ---------------------------------------------------------------------------
# host wrapper
# ---------------------------------------------------------------------------

def kernel(state_sequence, W1, b1, W2, b2, gamma, beta,
           We1, be1, We2, be2, We3, be3, Ws1, bs1, Ws2, bs2):
    global LAST_RESULTS
    from concourse.bass_utils import run_bass_kernel_spmd

    state_sequence = np.asarray(state_sequence, dtype=np.float32)
    W1 = np.asarray(W1, np.float32); b1 = np.asarray(b1, np.float32)
    W2 = np.asarray(W2, np.float32); b2 = np.asarray(b2, np.float32)
    gamma = np.asarray(gamma, np.float32); beta = np.asarray(beta, np.float32)
    We1 = np.asarray(We1, np.float32); be1 = np.asarray(be1, np.float32)
    We2 = np.asarray(We2, np.float32); be2 = np.asarray(be2, np.float32)
    We3 = np.asarray(We3, np.float32); be3 = np.asarray(be3, np.float32)
    Ws1 = np.asarray(Ws1, np.float32); bs1 = np.asarray(bs1, np.float32)
    Ws2 = np.asarray(Ws2, np.float32); bs2 = np.asarray(bs2, np.float32)

    idx = _node_indices()
    nodes = state_sequence[:, idx]                      # [B, N, D]

    def f16(a):
        return np.ascontiguousarray(a.astype(np.float16))

    def f32c(a):
        return np.ascontiguousarray(a.astype(np.float32))

    def blockdiag(w, k):
        m, n = w.shape
        out = np.zeros((m * k, n * k), np.float32)
        for i in range(k):
            out[i * m:(i + 1) * m, i * n:(i + 1) * n] = w
        return out

    parts16 = {
        "w1_0": W1[0:128], "w1_1": W1[128:256], "w1_2": W1[256:384],
        "w1_3": W1[384:512],
        "w2_0": W2[0:128], "w2_1": W2[128:256],
        "we1a": gamma[:, None] * We1[:128, :],
        "we1b": gamma[:, None] * We1[128:, :],
        "ws1a": gamma[:, None] * Ws1[:128, :],
        "ws1b": gamma[:, None] * Ws1[128:, :],
        "bd2we2": blockdiag(We2, 2),
        "bd4we3": blockdiag(We3, 4),
        "bd4ws2": blockdiag(Ws2, 4),
    }
    wb16 = np.zeros((128, W16_COLS), np.float16)
    for k, (o, w) in W16_OFF.items():
        wb16[:, o:o + w] = parts16[k].astype(np.float16)
    parts32 = {
        "b1_2": b1.reshape(2, 128).T, "b2c": b2.reshape(128, 1),
        "gammap": gamma.reshape(128, 1), "betap": beta.reshape(128, 1),
        "be1_2": np.tile(be1, 2).reshape(128, 1),
        "bs1_4": np.tile(bs1, 4).reshape(128, 1),
        "be2_4": np.tile(be2, 4).reshape(128, 1),
        "be3b": np.full((128, 1), float(be3[0])),
        "bs2b": np.full((128, 1), float(bs2[0])),
        # per-partition sigmoid affine: edge rows (q%32<4): sigmoid(x+be3);
        # strength rows (4<=q%32<8): sigmoid(2x+2*bs2) -> host maps 2y-1=tanh
        "sigb": np.array([[float(be3[0]) if (q % 32) < 4 else
                           2.0 * float(bs2[0])] for q in range(128)],
                         np.float64),
        "sigs": np.array([[1.0 if (q % 32) < 4 else 2.0] for q in range(128)],
                         np.float64),
        "beA": np.tile(0.5 * be1 + We1[:128].T @ beta, 2).reshape(128, 1),
        "beB": np.tile(0.5 * be1 + We1[128:].T @ beta, 2).reshape(128, 1),
        "bsA": np.tile(0.5 * bs1 + Ws1[:128].T @ beta, 4).reshape(128, 1),
        "bsB": np.tile(0.5 * bs1 + Ws1[128:].T @ beta, 4).reshape(128, 1),
    }
    wb32 = np.zeros((128, W32_COLS), np.float32)
    for k, v in parts32.items():
        o = W32_OFF[k]
        wb32[:, o:o + v.shape[1]] = v
    shared = {"wb16": wb16, "wb32": f32c(wb32)}

    in_maps = []
    for c in range(NCORES):
        xt = f16(nodes[c * BSH:(c + 1) * BSH].reshape(TOK, D).T)
        in_maps.append({"xt": xt, **shared})

    nc = _get_program()
    res = run_bass_kernel_spmd(nc, in_maps, core_ids=list(range(NCORES)))
    LAST_RESULTS = res

    nf = np.empty((B, N, DM), np.float32)
    adj = np.empty((B, N, N), np.float32)
    stg = np.empty((B, N, N), np.float32)
    mask = (1.0 - np.eye(N, dtype=np.float32))
    rows_adj = np.array([32 * (b // 4) + b % 4 for b in range(BSH)])
    rows_str = rows_adj + 4
    for c in range(NCORES):
        r = res.results[c]
        nf_c = r["nf_raw"].reshape(BSH, N, DM) * gamma + beta
        nf[c * BSH:(c + 1) * BSH] = nf_c
        pr = r["pairs"].astype(np.float32)
        adj[c * BSH:(c + 1) * BSH] = \
            pr[0][rows_adj].reshape(BSH, N, N).transpose(0, 2, 1) * mask
        stg[c * BSH:(c + 1) * BSH] = \
            pr[1][rows_adj].reshape(BSH, N, N).transpose(0, 2, 1) * mask
    return nf, adj, stg


# revision 26
# speedup vs baseline: 1.0403x; 1.0403x over previous
"""CausalGraphBuilder Trainium2 kernel.

Full inputs -> shard batch (128) over 8 NeuronCores (16 each) -> Bass kernel
(encoder MLP + LayerNorm + N^2 pair-MLP edge/strength heads) -> gather.

Self-contained: hardcodes B,S,D,N = 128,1024,512,64 and the sharding.
"""

import numpy as np

B, S, D, N = 128, 1024, 512, 64
DM = D // 4          # 128 node-feature dim after encoder
NCORES = 8
BSH = B // NCORES    # 16 batch per core
TOK = BSH * N        # 1024 tokens per core

# jnp.linspace(0, S-1, N).astype(int32) as evaluated by the reference in this
# environment (device cast rounds); recomputed at runtime when jax is available.
_FALLBACK_IDX = [0, 16, 32, 49, 65, 81, 97, 114, 130, 146, 162, 179, 195, 211,
                 227, 244, 260, 276, 292, 309, 325, 341, 357, 373, 390, 406,
                 422, 438, 455, 471, 487, 503, 520, 536, 552, 568, 585, 601,
                 617, 633, 650, 666, 682, 698, 714, 731, 747, 763, 779, 796,
                 812, 828, 844, 861, 877, 893, 909, 926, 942, 958, 974, 991,
                 1007, 1023]


def _node_indices():
    try:
        import jax.numpy as jnp
        idx = np.asarray(jnp.linspace(0.0, float(S - 1), N).astype(jnp.int32))
        if idx.shape == (N,):
            return idx.astype(np.int64)
    except Exception:
        pass
    return np.array(_FALLBACK_IDX, dtype=np.int64)


# ---------------------------------------------------------------------------
# device program
# ---------------------------------------------------------------------------

_PROGRAM_CACHE = {}
LAST_RESULTS = None  # BassKernelResults of the most recent run (for test.py)

# engine assignment knobs (tuned against the profile)
PE_E_TILES = ()             # E-groups built on PE (identity-matmul 2-pass)
PE_S_TILES = (1, 3)     # S-groups built on PE
EVAC_DVE_E = ()               # PE-built E tiles evacuated by DVE instead of ACT
EVAC_DVE_S = ()
GP_RELU = False               # DVE-built tiles: relu on GPSIMD (slow ucode!)
E2RELU_DVE = ()               # e2 cp-chunks (0..3) whose relu-evac runs on DVE



# packed-constant blob layouts (columns)
_W16_SECTIONS = [
    ("w1_0", 256), ("w1_1", 256), ("w1_2", 256), ("w1_3", 256),
    ("w2_0", 128), ("w2_1", 128),
    ("we1a", 64), ("we1b", 64), ("ws1a", 32), ("ws1b", 32),
    ("bd2we2", 64), ("bd4we3", 4), ("bd4ws2", 4),
]
W16_OFF = {}
_o = 0
for _k, _w in _W16_SECTIONS:
    W16_OFF[_k] = (_o, _w)
    _o += _w
W16_COLS = _o
_W32_SECTIONS = [("b1_2", 2), ("b2c", 1), ("gammap", 1), ("betap", 1),
                 ("be1_2", 1), ("bs1_4", 1), ("be2_4", 1), ("be3b", 1),
                 ("bs2b", 1), ("sigb", 1), ("sigs", 1), ("beA", 1),
                 ("beB", 1), ("bsA", 1), ("bsB", 1)]
W32_OFF = {}
_o = 0
for _k, _w in _W32_SECTIONS:
    W32_OFF[_k] = _o
    _o += _w
W32_COLS = _o

def _patched_tile_context(nc):
    """TileContext whose tail drain never carries >1 sync wait (this walrus
    build rejects multi-wait CTRL instructions)."""
    import concourse.mybir as mybir
    import concourse.tile as tile
    from concourse.vector_clock import ScopedClock

    class TileContextP(tile.TileContext):
        def _drain_and_barrier(self, tick_clock, wait_clock):
            drain_inst = self.nc.sync.drain()
            wait_clock.add_sem_waits(
                drain_inst.ins, ScopedClock({None: tick_clock.global_clock})
            )
            si = drain_inst.ins.sync_info
            if si is not None and si.on_wait is not None and len(si.on_wait) > 1:
                waits = list(si.on_wait)
                si.on_wait = waits[:1]
                for w in waits[1:]:
                    extra = self.nc.sync.drain()
                    extra.ins.sync_info = mybir.SyncInfo(on_wait=[w], on_update=[])
            self.nc.all_engine_barrier()
            popped = self.nc._tile_sem_poison_stack.pop()
            assert popped is self._sem_poison
            self.nc.clear_and_free_semaphores(list(self.sems.allocated().values()))

    return TileContextP(nc)


def _split_multiwait(nc):
    """This walrus build accepts at most ONE sync wait per instruction; hoist
    extra waits into single-wait NoOps on the same engine just before."""
    import concourse.mybir as mybir

    n_split = 0
    for f in nc.m.functions:
        for bb in f.blocks:
            insts = list(bb.instructions)
            out = []
            for ins in insts:
                si = ins.sync_info
                if si is not None and si.on_wait is not None and len(si.on_wait) > 1:
                    waits = list(si.on_wait)
                    for w in waits[:-1]:
                        nop = mybir.InstNoOp(
                            name=f"{ins.name}-w{n_split}",
                            engine=ins.engine,
                            bass_nofuse=True,
                            sync_info=mybir.SyncInfo(on_wait=[w], on_update=[]),
                        )
                        out.append(nop)
                        n_split += 1
                    si.on_wait = waits[-1:]
                out.append(ins)
            if n_split:
                bb.instructions = out
    return n_split


def _build_program():
    import concourse.bass as bass
    import concourse.mybir as mybir
    from concourse.masks import make_identity

    f16 = mybir.dt.float16
    f32 = mybir.dt.float32
    AF = mybir.ActivationFunctionType
    OP = mybir.AluOpType

    nc = bass.Bass()

    # ---- DRAM I/O ----
    xt_d = nc.dram_tensor("xt", [D, TOK], f16, kind="ExternalInput")
    wb16_d = nc.dram_tensor("wb16", [128, W16_COLS], f16, kind="ExternalInput")
    wb32_d = nc.dram_tensor("wb32", [128, W32_COLS], f32, kind="ExternalInput")

    nf_d = nc.dram_tensor("nf_raw", [TOK, DM], f32, kind="ExternalOutput")
    pairs_d = nc.dram_tensor("pairs", [2, 128, N * N], f16, kind="ExternalOutput")

    with _patched_tile_context(nc) as tc:
        from contextlib import ExitStack

        with ExitStack() as ctx:
            consts = ctx.enter_context(tc.tile_pool(name="consts", bufs=1))
            persist = ctx.enter_context(tc.tile_pool(name="persist", bufs=1))

            # ---- load constants ----
            def load(pool, name, dram, shape, dtype):
                t = pool.tile(shape, dtype, tag=name, name=name)
                nc.sync.dma_start(out=t[:], in_=dram[:])
                return t

            wb32 = load(consts, "wb32", wb32_d, [128, W32_COLS], f32)
            wb16 = load(consts, "wb16", wb16_d, [128, W16_COLS], f16)
            dummy_sig = consts.tile([128, 1], f32, tag="dsig", name="dummy_sig")
            nc.scalar.activation(out=dummy_sig[:], in_=wb32[:, 0:1],
                                 func=AF.Sigmoid)
            xt_sb4 = consts.tile([128, 4, TOK], f16, tag="xt", name="xt_sb4")
            for c in range(4):
                eng = nc.gpsimd if c % 2 == 0 else nc.sync
                eng.dma_start(
                    out=xt_sb4[:, c, :], in_=xt_d[c * 128:(c + 1) * 128, :])
            xt_sb = [xt_sb4[:, c, :] for c in range(4)]

            def c16(key):
                o, w = W16_OFF[key]
                return wb16[:, o:o + w]

            def c32(key):
                o = W32_OFF[key]
                return wb32[:, o:o + 1]

            w1_sb = [c16(f"w1_{c}") for c in range(4)]
            w2_sb = [c16(f"w2_{c}") for c in range(2)]
            we1a_sb = c16("we1a"); we1b_sb = c16("we1b")
            ws1a_sb = c16("ws1a"); ws1b_sb = c16("ws1b")
            bd2we2_sb = c16("bd2we2")
            bd4we3_sb = c16("bd4we3"); bd4ws2_sb = c16("bd4ws2")
            b1_sb = wb32[:, W32_OFF["b1_2"]:W32_OFF["b1_2"] + 2]
            b2_sb = c32("b2c"); gp_sb = c32("gammap"); bp_sb = c32("betap")
            be1_sb = c32("be1_2"); bs1_sb = c32("bs1_4"); be2_sb = c32("be2_4")
            be3_sb = c32("be3b"); bs2_sb = c32("bs2b")
            sigb_sb = c32("sigb"); sigs_sb = c32("sigs")
            beA_sb = c32("beA"); beB_sb = c32("beB")
            bsA_sb = c32("bsA"); bsB_sb = c32("bsB")

            eps_sb = consts.tile([128, 1], f32, tag="eps")
            nc.vector.memset(eps_sb[:], 1e-5)

            idf32 = consts.tile([128, 128], f32, tag="idf32")
            make_identity(nc, idf32[:])
            idf16 = consts.tile([128, 128], f16, tag="idf16")
            make_identity(nc, idf16[:])

            h1t = [persist.tile([128, TOK], f16, tag=f"h1t{m}", name=f"h1t{m}") for m in range(2)]
            ht = persist.tile([128, TOK], f32, tag="ht")
            nfraw = persist.tile([128, 8, 128], f32, tag="nfraw")
            nft = persist.tile([128, TOK], f16, tag="nft")
            ab_all = persist.tile([128, 8, 192], f16, tag="ab_all")
            ab = [ab_all[:, g, :] for g in range(8)]
            apbp_all = persist.tile([128, 4, 192], f16, tag="apbp_all")
            apbp = [apbp_all[:, gr, :] for gr in range(4)]
            e2r = [persist.tile([128, N * N], f16, tag=f"e2r{u}", name=f"e2r{u}") for u in range(4)]
            s1r = [persist.tile([128, N * N], f16, tag=f"s1r{gr}", name=f"s1r{gr}") for gr in range(4)]

            # ================= encoder =================
            with tc.tile_pool(name="ps_enc", bufs=2, space="PSUM") as ps_enc:
                for m in range(2):
                    for n2 in range(2):
                        ps = ps_enc.tile([128, 512], f32, tag="enc", name="ps_enc_t")
                        for c in range(4):
                            nc.tensor.matmul(
                                ps[:],
                                lhsT=w1_sb[c][:, m * 128:(m + 1) * 128],
                                rhs=xt_sb[c][:, n2 * 512:(n2 + 1) * 512],
                                start=(c == 0), stop=(c == 3),
                            )
                        nc.scalar.activation(
                            out=h1t[m][:, n2 * 512:(n2 + 1) * 512], in_=ps[:],
                            func=AF.Relu, bias=b1_sb[:, m:m + 1], scale=1.0,
                        )
                for n2 in range(2):
                    ps = ps_enc.tile([128, 512], f32, tag="enc", name="ps_enc_t")
                    for c2 in range(2):
                        nc.tensor.matmul(
                            ps[:], lhsT=w2_sb[c2][:],
                            rhs=h1t[c2][:, n2 * 512:(n2 + 1) * 512],
                            start=(c2 == 0), stop=(c2 == 1),
                        )
                    nc.scalar.activation(
                        out=ht[:, n2 * 512:(n2 + 1) * 512], in_=ps[:],
                        func=AF.Identity, bias=b2_sb[:, 0:1], scale=1.0,
                    )

            # ================= layernorm (token-major) =================
            # sqrt-free: rstd = Newton-rsqrt from 1/x seed (keeps a single
            # ACT table set resident for the whole kernel)
            mu_all = persist.tile([128, 8], f32, tag="mu_all")
            rstd_all = persist.tile([128, 8], f32, tag="rstd_all")
            ve_all = persist.tile([128, 8], f32, tag="ve_all")
            nmr_all = persist.tile([128, 8], f32, tag="nmr_all")
            with tc.tile_pool(name="ps_tr", bufs=1, space="PSUM") as ps_tr, \
                 tc.tile_pool(name="ln_tmp", bufs=4) as ln_tmp:
                pstok = ps_tr.tile([128, 1024], f32, tag="htok", name="pstok")
                for t in range(8):
                    nc.tensor.transpose(pstok[:, t * 128:(t + 1) * 128],
                                        ht[:, t * 128:(t + 1) * 128], idf32[:])
                st6 = ln_tmp.tile([128, 8, 6], f32, tag="st6", name="st6")
                p3 = pstok[:].rearrange("p (t f) -> p t f", t=8)
                for t in range(8):
                    nc.vector.bn_stats(out=st6[:, t, :], in_=p3[:, t, :])
                mv3 = ln_tmp.tile([128, 8, 2], f32, tag="mv3", name="mv3")
                for t in range(8):
                    nc.vector.bn_aggr(out=mv3[:, t, :], in_=st6[:, t, :])
                nc.vector.tensor_copy(mu_all[:], mv3[:, :, 0])
                # rstd = rsqrt(ve + eps), Newton x3 from linear-in-1/x seed
                nc.vector.tensor_scalar_add(out=ve_all[:], in0=mv3[:, :, 1],
                                            scalar1=1e-5)
                rr = ln_tmp.tile([128, 8], f32, tag="rr", name="rr")
                nc.vector.reciprocal(out=rr[:], in_=ve_all[:])
                nc.vector.tensor_scalar(out=rstd_all[:], in0=rr[:],
                                        scalar1=0.345, scalar2=0.53,
                                        op0=OP.mult, op1=OP.add)
                t2 = ln_tmp.tile([128, 8], f32, tag="t2", name="t2")
                for _ in range(3):
                    nc.vector.tensor_mul(t2[:], rstd_all[:], rstd_all[:])
                    nc.vector.tensor_mul(t2[:], t2[:], ve_all[:])
                    nc.vector.tensor_scalar(out=t2[:], in0=t2[:],
                                            scalar1=-0.5, scalar2=1.5,
                                            op0=OP.mult, op1=OP.add)
                    nc.vector.tensor_mul(rstd_all[:], rstd_all[:], t2[:])
                nc.vector.tensor_mul(nmr_all[:], mu_all[:], rstd_all[:])
                nc.vector.tensor_scalar_mul(out=nmr_all[:], in0=nmr_all[:],
                                            scalar1=-1.0)
                for t in range(8):
                    nc.scalar.activation(
                        out=nfraw[:, t, :], in_=pstok[:, t * 128:(t + 1) * 128],
                        func=AF.Identity, bias=nmr_all[:, t:t + 1],
                        scale=rstd_all[:, t:t + 1])
                nc.sync.dma_start(
                    out=nf_d[:].rearrange("(t p) f -> p t f", t=8, p=128),
                    in_=nfraw[:])

                # nfT (feature-major) with gamma/beta applied per-partition
                psn = ps_tr.tile([128, 1024], f32, tag="nft_ps", name="psn")
                for t in range(8):
                    nc.tensor.transpose(psn[:, t * 128:(t + 1) * 128],
                                        nfraw[:, t, :], idf32[:])
                nc.scalar.activation(out=nft[:], in_=psn[:], func=AF.Copy)

            # ================= pair projections =================
            # Fat matmuls with strided token-gather APs: for each partition
            # parity bb, one matmul gathers the i-side (A) columns of all its
            # groups; the j-side (B) is emitted with duplicated columns for
            # the 2x-mode pair build. Bias be1/bs1 is added as bias/2 on both
            # the A and B evacuations.
            with tc.tile_pool(name="ps_proj", bufs=1, space="PSUM") as ps_proj:
                psE = ps_proj.tile([128, 1536], f32, tag="proj_e",
                                   name="psE")
                nftv = nft[:]
                for bb in range(2):
                    # A: [64f, 8 groups x 64 cols]; tokens of b = 2g + bb
                    rhs_a = bass.AP(
                        tensor=nftv.tensor, offset=nftv.offset + bb * N,
                        ap=[list(nftv.ap[0]), [2 * N, 8], [1, N]])
                    nc.tensor.matmul(
                        psE[64 * bb:64 * bb + 64, 0:512], lhsT=we1a_sb[:],
                        rhs=rhs_a, start=True, stop=True,
                        tile_position=(0, 64 * bb))
                    # B dup: [64f, 8 groups x 128 cols] (two 512-col mms)
                    for hf in range(2):
                        rhs_b = bass.AP(
                            tensor=nftv.tensor,
                            offset=nftv.offset + bb * N + hf * 4 * 2 * N,
                            ap=[list(nftv.ap[0]), [2 * N, 4], [1, N], [0, 2]])
                        nc.tensor.matmul(
                            psE[64 * bb:64 * bb + 64,
                                512 + hf * 512:512 + (hf + 1) * 512],
                            lhsT=we1b_sb[:], rhs=rhs_b, start=True, stop=True,
                            tile_position=(0, 64 * bb))
                nc.scalar.activation(
                    out=ab_all[:, :, 0:64],
                    in_=psE[:, 0:512].rearrange("p (g n) -> p g n", g=8),
                    func=AF.Identity, bias=beA_sb[:, 0:1], scale=1.0)
                nc.scalar.activation(
                    out=ab_all[:, :, 64:192],
                    in_=psE[:, 512:1536].rearrange("p (g n) -> p g n", g=8),
                    func=AF.Identity, bias=beB_sb[:, 0:1], scale=1.0)

                psS = ps_proj.tile([128, 1024], f32, tag="proj_s", name="psS")
                for bb in range(4):
                    rhs_a = bass.AP(
                        tensor=nftv.tensor, offset=nftv.offset + bb * N,
                        ap=[list(nftv.ap[0]), [4 * N, 4], [1, N]])
                    nc.tensor.matmul(
                        psS[32 * bb:32 * bb + 32, 0:256], lhsT=ws1a_sb[:],
                        rhs=rhs_a, start=True, stop=True,
                        tile_position=(0, 32 * bb))
                    rhs_b = bass.AP(
                        tensor=nftv.tensor, offset=nftv.offset + bb * N,
                        ap=[list(nftv.ap[0]), [4 * N, 4], [1, N], [0, 2]])
                    nc.tensor.matmul(
                        psS[32 * bb:32 * bb + 32, 512:1024], lhsT=ws1b_sb[:],
                        rhs=rhs_b, start=True, stop=True,
                        tile_position=(0, 32 * bb))
                nc.scalar.activation(
                    out=apbp_all[:, :, 0:64],
                    in_=psS[:, 0:256].rearrange("p (g n) -> p g n", g=4),
                    func=AF.Identity, bias=bsA_sb[:, 0:1], scale=1.0)
                nc.scalar.activation(
                    out=apbp_all[:, :, 64:192],
                    in_=psS[:, 512:1024].rearrange("p (g n) -> p g n", g=4),
                    func=AF.Identity, bias=bsB_sb[:, 0:1], scale=1.0)

            # ================= N^2 builds + edge layer2 =================
            # E-path: per u (2 edge groups): DVE builds -> PE e2 -> ACT evac,
            # with finals-half0 matmuls interleaved (PSUM: e2 4 banks +
            # finals-h0 4 banks). S-path runs at the tail, overlapping
            # E-finals/sigmoid.
            relu_act_counter = [0]

            def build_dve(out3, src):
                out4 = out3.rearrange("p j (i2 ii) -> p j i2 ii", ii=2)
                a4 = src[:, None, 0:64].broadcast_to((128, N, 64)) \
                    .rearrange("p j (i2 ii) -> p j i2 ii", ii=2)
                b4 = src[:, 64:192].rearrange("p (j ii) -> p j ii", ii=2)[
                    :, :, None, :].broadcast_to((128, N, 32, 2))
                k = relu_act_counter[0]
                relu_act_counter[0] += 1
                flat = out3.rearrange("p i j -> p (i j)")
                for h in range(2):
                    nc.vector.tensor_add(out4[:, 32 * h:32 * h + 32],
                                         b4[:, 32 * h:32 * h + 32],
                                         a4[:, 32 * h:32 * h + 32])
                    hf = flat[:, h * 2048:(h + 1) * 2048]
                    if k % 4 == 1:    # every 4th tile relu on ACT
                        nc.scalar.activation(out=hf, in_=hf, func=AF.Relu)
                    else:
                        nc.vector.tensor_scalar_max(out=hf, in0=hf,
                                                    scalar1=0.0)

            adj_t = [None, None]
            str_t = [None, None]
            with tc.tile_pool(name="e1pool", bufs=4) as e1pool, \
                 tc.tile_pool(name="ps_e2", bufs=2, space="PSUM") as ps_e2, \
                 tc.tile_pool(name="ps_finE0", bufs=1, space="PSUM") as ps_finE0, \
                 tc.tile_pool(name="outstage", bufs=1) as outstage:
                finE0 = ps_finE0.tile([128, 2048], f32, tag="fe0", name="finE0")
                for u in range(4):
                    e1t = []
                    for g in (2 * u, 2 * u + 1):
                        e1 = e1pool.tile([128, N, N], f16, tag="e1",
                                         name=f"e1_{g}")
                        build_dve(e1[:], ab[g])
                        e1t.append(e1[:].rearrange("p i j -> p (i j)"))
                    for cp in range(4):
                        ps = ps_e2.tile([128, 1024], f32, tag="e2",
                                        name="ps_e2_t")
                        for hf in range(2):
                            c0 = cp * 1024 + hf * 512
                            nc.tensor.matmul(
                                ps[0:64, hf * 512:(hf + 1) * 512],
                                lhsT=bd2we2_sb[:], rhs=e1t[0][:, c0:c0 + 512],
                                start=True, stop=True, tile_position=(0, 0),
                            )
                            nc.tensor.matmul(
                                ps[64:128, hf * 512:(hf + 1) * 512],
                                lhsT=bd2we2_sb[:], rhs=e1t[1][:, c0:c0 + 512],
                                start=True, stop=True, tile_position=(0, 64),
                            )
                        nc.scalar.activation(
                            out=e2r[u][:, cp * 1024:(cp + 1) * 1024], in_=ps[:],
                            func=AF.Relu, bias=be2_sb[:, 0:1], scale=1.0,
                        )
                    for ch in range(4):     # finals half 0 for this u
                        nc.tensor.matmul(
                            finE0[32 * u:32 * u + 4, ch * 512:(ch + 1) * 512],
                            lhsT=bd4we3_sb[:], rhs=e2r[u][:, ch * 512:
                                                          (ch + 1) * 512],
                            start=True, stop=True, tile_position=(0, 32 * u),
                        )
                    build_dve(s1r[u][:].rearrange("p (i j) -> p i j", i=N),
                              apbp[u])
                adj_t[0] = outstage.tile([128, 2048], f16, tag="adj0",
                                         name="adj_t0")
                nc.scalar.activation(out=adj_t[0][:], in_=finE0[:],
                                     func=AF.Sigmoid, bias=be3_sb[:, 0:1],
                                     scale=1.0)
                nc.sync.dma_start(out=pairs_d[0, :, 0:1024],
                                  in_=adj_t[0][:, 0:1024])
                nc.gpsimd.dma_start(out=pairs_d[0, :, 1024:2048],
                                    in_=adj_t[0][:, 1024:2048])

            # tail: S finals h0 -> E finals h1 -> S finals h1
            with tc.tile_pool(name="ps_fin2", bufs=2, space="PSUM") as ps_fin2, \
                 tc.tile_pool(name="outstage2", bufs=2) as outstage2:
                def head_mms(ps, lhsT, rhs_tiles, c0):
                    for ch in range(4):
                        for k in range(4):
                            nc.tensor.matmul(
                                ps[32 * k:32 * k + 4, ch * 512:(ch + 1) * 512],
                                lhsT=lhsT,
                                rhs=rhs_tiles[k][:, c0 + ch * 512:
                                                 c0 + (ch + 1) * 512],
                                start=True, stop=True,
                                tile_position=(0, 32 * k),
                            )

                s1aps = [s1r[gr][:] for gr in range(4)]
                e2aps = [e2r[u][:] for u in range(4)]
                ps_s0 = ps_fin2.tile([128, 2048], f32, tag="fin",
                                     name="ps_s0")
                head_mms(ps_s0, bd4ws2_sb[:], s1aps, 0)
                st0 = outstage2.tile([128, 2048], f16, tag="o2", name="st0")
                nc.scalar.activation(out=st0[:], in_=ps_s0[:], func=AF.Tanh,
                                     bias=bs2_sb[:, 0:1], scale=1.0)
                nc.sync.dma_start(out=pairs_d[1, :, 0:1024],
                                  in_=st0[:, 0:1024])
                nc.gpsimd.dma_start(out=pairs_d[1, :, 1024:2048],
                                    in_=st0[:, 1024:2048])

                ps_e1 = ps_fin2.tile([128, 2048], f32, tag="fin",
                                     name="ps_e1f")
                head_mms(ps_e1, bd4we3_sb[:], e2aps, 2048)
                at1 = outstage2.tile([128, 2048], f16, tag="o2", name="at1")
                nc.scalar.activation(out=at1[:], in_=ps_e1[:], func=AF.Sigmoid,
                                     bias=be3_sb[:, 0:1], scale=1.0)
                nc.sync.dma_start(out=pairs_d[0, :, 2048:3072],
                                  in_=at1[:, 0:1024])
                nc.gpsimd.dma_start(out=pairs_d[0, :, 3072:4096],
                                    in_=at1[:, 1024:2048])

                ps_s1 = ps_fin2.tile([128, 2048], f32, tag="fin",
                                     name="ps_s1f")
                head_mms(ps_s1, bd4ws2_sb[:], s1aps, 2048)
                st1 = outstage2.tile([128, 2048], f16, tag="o2", name="st1")
                nc.scalar.activation(out=st1[:], in_=ps_s1[:], func=AF.Tanh,
                                     bias=bs2_sb[:, 0:1], scale=1.0)
                nc.sync.dma_start(out=pairs_d[1, :, 2048:3072],
                                  in_=st1[:, 0:1024])
                nc.gpsimd.dma_start(out=pairs_d[1, :, 3072:4096],
                                    in_=st1[:, 1024:2048])

    _split_multiwait(nc)
    return nc


def _get_program():
    if "nc" not in _PROGRAM_CACHE:
        _PROGRAM_CACHE["nc"] = _build_program()
    return _PROGRAM_CACHE["nc"]


# ---------------------------------------------------------------------------
# host wrapper
# ---------------------------------------------------------------------------

def kernel(state_sequence, W1, b1, W2, b2, gamma, beta,
           We1, be1, We2, be2, We3, be3, Ws1, bs1, Ws2, bs2):
    global LAST_RESULTS
    from concourse.bass_utils import run_bass_kernel_spmd

    state_sequence = np.asarray(state_sequence, dtype=np.float32)
    W1 = np.asarray(W1, np.float32); b1 = np.asarray(b1, np.float32)
    W2 = np.asarray(W2, np.float32); b2 = np.asarray(b2, np.float32)
    gamma = np.asarray(gamma, np.float32); beta = np.asarray(beta, np.float32)
    We1 = np.asarray(We1, np.float32); be1 = np.asarray(be1, np.float32)
    We2 = np.asarray(We2, np.float32); be2 = np.asarray(be2, np.float32)
    We3 = np.asarray(We3, np.float32); be3 = np.asarray(be3, np.float32)
    Ws1 = np.asarray(Ws1, np.float32); bs1 = np.asarray(bs1, np.float32)
    Ws2 = np.asarray(Ws2, np.float32); bs2 = np.asarray(bs2, np.float32)

    idx = _node_indices()
    nodes = state_sequence[:, idx]                      # [B, N, D]

    def f16(a):
        return np.ascontiguousarray(a.astype(np.float16))

    def f32c(a):
        return np.ascontiguousarray(a.astype(np.float32))

    def blockdiag(w, k):
        m, n = w.shape
        out = np.zeros((m * k, n * k), np.float32)
        for i in range(k):
            out[i * m:(i + 1) * m, i * n:(i + 1) * n] = w
        return out

    parts16 = {
        "w1_0": W1[0:128], "w1_1": W1[128:256], "w1_2": W1[256:384],
        "w1_3": W1[384:512],
        "w2_0": W2[0:128], "w2_1": W2[128:256],
        "we1a": gamma[:, None] * We1[:128, :],
        "we1b": gamma[:, None] * We1[128:, :],
        "ws1a": gamma[:, None] * Ws1[:128, :],
        "ws1b": gamma[:, None] * Ws1[128:, :],
        "bd2we2": blockdiag(We2, 2),
        "bd4we3": blockdiag(We3, 4),
        "bd4ws2": blockdiag(Ws2, 4),
    }
    wb16 = np.zeros((128, W16_COLS), np.float16)
    for k, (o, w) in W16_OFF.items():
        wb16[:, o:o + w] = parts16[k].astype(np.float16)
    parts32 = {
        "b1_2": b1.reshape(2, 128).T, "b2c": b2.reshape(128, 1),
        "gammap": gamma.reshape(128, 1), "betap": beta.reshape(128, 1),
        "be1_2": np.tile(be1, 2).reshape(128, 1),
        "bs1_4": np.tile(bs1, 4).reshape(128, 1),
        "be2_4": np.tile(be2, 4).reshape(128, 1),
        "be3b": np.full((128, 1), float(be3[0])),
        "bs2b": np.full((128, 1), float(bs2[0])),
        # per-partition sigmoid affine: edge rows (q%32<4): sigmoid(x+be3);
        # strength rows (4<=q%32<8): sigmoid(2x+2*bs2) -> host maps 2y-1=tanh
        "sigb": np.array([[float(be3[0]) if (q % 32) < 4 else
                           2.0 * float(bs2[0])] for q in range(128)],
                         np.float64),
        "sigs": np.array([[1.0 if (q % 32) < 4 else 2.0] for q in range(128)],
                         np.float64),
        "beA": np.tile(0.5 * be1 + We1[:128].T @ beta, 2).reshape(128, 1),
        "beB": np.tile(0.5 * be1 + We1[128:].T @ beta, 2).reshape(128, 1),
        "bsA": np.tile(0.5 * bs1 + Ws1[:128].T @ beta, 4).reshape(128, 1),
        "bsB": np.tile(0.5 * bs1 + Ws1[128:].T @ beta, 4).reshape(128, 1),
    }
    wb32 = np.zeros((128, W32_COLS), np.float32)
    for k, v in parts32.items():
        o = W32_OFF[k]
        wb32[:, o:o + v.shape[1]] = v
    shared = {"wb16": wb16, "wb32": f32c(wb32)}

    in_maps = []
    for c in range(NCORES):
        xt = f16(nodes[c * BSH:(c + 1) * BSH].reshape(TOK, D).T)
        in_maps.append({"xt": xt, **shared})

    nc = _get_program()
    res = run_bass_kernel_spmd(nc, in_maps, core_ids=list(range(NCORES)))
    LAST_RESULTS = res

    nf = np.empty((B, N, DM), np.float32)
    adj = np.empty((B, N, N), np.float32)
    stg = np.empty((B, N, N), np.float32)
    mask = (1.0 - np.eye(N, dtype=np.float32))
    rows_adj = np.array([32 * (b // 4) + b % 4 for b in range(BSH)])
    rows_str = rows_adj + 4
    for c in range(NCORES):
        r = res.results[c]
        nf_c = r["nf_raw"].reshape(BSH, N, DM) * gamma + beta
        nf[c * BSH:(c + 1) * BSH] = nf_c
        pr = r["pairs"].astype(np.float32)
        adj[c * BSH:(c + 1) * BSH] = \
            pr[0][rows_adj].reshape(BSH, N, N).transpose(0, 2, 1) * mask
        stg[c * BSH:(c + 1) * BSH] = \
            pr[1][rows_adj].reshape(BSH, N, N).transpose(0, 2, 1) * mask
    return nf, adj, stg


# revision 27
# speedup vs baseline: 1.2030x; 1.1565x over previous
"""CausalGraphBuilder Trainium2 kernel.

Full inputs -> shard batch (128) over 8 NeuronCores (16 each) -> Bass kernel
(encoder MLP + LayerNorm + N^2 pair-MLP edge/strength heads) -> gather.

Self-contained: hardcodes B,S,D,N = 128,1024,512,64 and the sharding.
"""

import numpy as np

B, S, D, N = 128, 1024, 512, 64
DM = D // 4          # 128 node-feature dim after encoder
NCORES = 8
BSH = B // NCORES    # 16 batch per core
TOK = BSH * N        # 1024 tokens per core

# jnp.linspace(0, S-1, N).astype(int32) as evaluated by the reference in this
# environment (device cast rounds); recomputed at runtime when jax is available.
_FALLBACK_IDX = [0, 16, 32, 49, 65, 81, 97, 114, 130, 146, 162, 179, 195, 211,
                 227, 244, 260, 276, 292, 309, 325, 341, 357, 373, 390, 406,
                 422, 438, 455, 471, 487, 503, 520, 536, 552, 568, 585, 601,
                 617, 633, 650, 666, 682, 698, 714, 731, 747, 763, 779, 796,
                 812, 828, 844, 861, 877, 893, 909, 926, 942, 958, 974, 991,
                 1007, 1023]


def _node_indices():
    try:
        import jax.numpy as jnp
        idx = np.asarray(jnp.linspace(0.0, float(S - 1), N).astype(jnp.int32))
        if idx.shape == (N,):
            return idx.astype(np.int64)
    except Exception:
        pass
    return np.array(_FALLBACK_IDX, dtype=np.int64)


# ---------------------------------------------------------------------------
# device program
# ---------------------------------------------------------------------------

_PROGRAM_CACHE = {}
LAST_RESULTS = None  # BassKernelResults of the most recent run (for test.py)

# engine assignment knobs (tuned against the profile)
PE_E_TILES = ()             # E-groups built on PE (identity-matmul 2-pass)
PE_S_TILES = (1, 3)     # S-groups built on PE
EVAC_DVE_E = ()               # PE-built E tiles evacuated by DVE instead of ACT
EVAC_DVE_S = ()
GP_RELU = False               # DVE-built tiles: relu on GPSIMD (slow ucode!)
E2RELU_DVE = ()               # e2 cp-chunks (0..3) whose relu-evac runs on DVE



# packed-constant blob layouts (columns)
_W16_SECTIONS = [
    ("w1_0", 256), ("w1_1", 256), ("w1_2", 256), ("w1_3", 256),
    ("w2_0", 128), ("w2_1", 128),
    ("we1a", 64), ("we1b", 64), ("ws1a", 32), ("ws1b", 32),
    ("bd2we2", 64), ("bd4we3", 4), ("bd4ws2", 4),
]
W16_OFF = {}
_o = 0
for _k, _w in _W16_SECTIONS:
    W16_OFF[_k] = (_o, _w)
    _o += _w
W16_COLS = _o
_W32_SECTIONS = [("b1_2", 2), ("b2c", 1), ("gammap", 1), ("betap", 1),
                 ("be1_2", 1), ("bs1_4", 1), ("be2_4", 1), ("be3b", 1),
                 ("bs2b", 1), ("sigb", 1), ("sigs", 1), ("beA", 1),
                 ("beB", 1), ("bsA", 1), ("bsB", 1)]
W32_OFF = {}
_o = 0
for _k, _w in _W32_SECTIONS:
    W32_OFF[_k] = _o
    _o += _w
W32_COLS = _o

def _patched_tile_context(nc):
    """TileContext whose tail drain never carries >1 sync wait (this walrus
    build rejects multi-wait CTRL instructions)."""
    import concourse.mybir as mybir
    import concourse.tile as tile
    from concourse.vector_clock import ScopedClock

    class TileContextP(tile.TileContext):
        def _drain_and_barrier(self, tick_clock, wait_clock):
            drain_inst = self.nc.sync.drain()
            wait_clock.add_sem_waits(
                drain_inst.ins, ScopedClock({None: tick_clock.global_clock})
            )
            si = drain_inst.ins.sync_info
            if si is not None and si.on_wait is not None and len(si.on_wait) > 1:
                waits = list(si.on_wait)
                si.on_wait = waits[:1]
                for w in waits[1:]:
                    extra = self.nc.sync.drain()
                    extra.ins.sync_info = mybir.SyncInfo(on_wait=[w], on_update=[])
            self.nc.all_engine_barrier()
            popped = self.nc._tile_sem_poison_stack.pop()
            assert popped is self._sem_poison
            self.nc.clear_and_free_semaphores(list(self.sems.allocated().values()))

    return TileContextP(nc)


def _split_multiwait(nc):
    """This walrus build accepts at most ONE sync wait per instruction; hoist
    extra waits into single-wait NoOps on the same engine just before."""
    import concourse.mybir as mybir

    n_split = 0
    for f in nc.m.functions:
        for bb in f.blocks:
            insts = list(bb.instructions)
            out = []
            for ins in insts:
                si = ins.sync_info
                if si is not None and si.on_wait is not None and len(si.on_wait) > 1:
                    waits = list(si.on_wait)
                    for w in waits[:-1]:
                        nop = mybir.InstNoOp(
                            name=f"{ins.name}-w{n_split}",
                            engine=ins.engine,
                            bass_nofuse=True,
                            sync_info=mybir.SyncInfo(on_wait=[w], on_update=[]),
                        )
                        out.append(nop)
                        n_split += 1
                    si.on_wait = waits[-1:]
                out.append(ins)
            if n_split:
                bb.instructions = out
    return n_split


def _build_program():
    import concourse.bass as bass
    import concourse.mybir as mybir
    from concourse.masks import make_identity

    f16 = mybir.dt.float16
    f32 = mybir.dt.float32
    AF = mybir.ActivationFunctionType
    OP = mybir.AluOpType

    nc = bass.Bass()

    # ---- DRAM I/O ----
    xt_d = nc.dram_tensor("xt", [D, TOK], f16, kind="ExternalInput")
    wb16_d = nc.dram_tensor("wb16", [128, W16_COLS], f16, kind="ExternalInput")
    wb32_d = nc.dram_tensor("wb32", [128, W32_COLS], f32, kind="ExternalInput")

    nf_d = nc.dram_tensor("nf_raw", [TOK, DM], f32, kind="ExternalOutput")
    pairs_d = nc.dram_tensor("pairs", [2, 128, N * N], f16, kind="ExternalOutput")

    with _patched_tile_context(nc) as tc:
        from contextlib import ExitStack

        with ExitStack() as ctx:
            consts = ctx.enter_context(tc.tile_pool(name="consts", bufs=1))
            persist = ctx.enter_context(tc.tile_pool(name="persist", bufs=1))

            # ---- load constants ----
            def load(pool, name, dram, shape, dtype):
                t = pool.tile(shape, dtype, tag=name, name=name)
                nc.sync.dma_start(out=t[:], in_=dram[:])
                return t

            wb32 = load(consts, "wb32", wb32_d, [128, W32_COLS], f32)
            wb16 = load(consts, "wb16", wb16_d, [128, W16_COLS], f16)
            dummy_sig = consts.tile([128, 1], f32, tag="dsig", name="dummy_sig")
            nc.scalar.activation(out=dummy_sig[:], in_=wb32[:, 0:1],
                                 func=AF.Sigmoid)
            xt_sb4 = consts.tile([128, 4, TOK], f16, tag="xt", name="xt_sb4")
            for c in range(4):
                eng = nc.sync if c % 2 == 0 else nc.gpsimd
                eng.dma_start(
                    out=xt_sb4[:, c, :], in_=xt_d[c * 128:(c + 1) * 128, :])
            xt_sb = [xt_sb4[:, c, :] for c in range(4)]

            def c16(key):
                o, w = W16_OFF[key]
                return wb16[:, o:o + w]

            def c32(key):
                o = W32_OFF[key]
                return wb32[:, o:o + 1]

            w1_sb = [c16(f"w1_{c}") for c in range(4)]
            w2_sb = [c16(f"w2_{c}") for c in range(2)]
            we1a_sb = c16("we1a"); we1b_sb = c16("we1b")
            ws1a_sb = c16("ws1a"); ws1b_sb = c16("ws1b")
            bd2we2_sb = c16("bd2we2")
            bd4we3_sb = c16("bd4we3"); bd4ws2_sb = c16("bd4ws2")
            b1_sb = wb32[:, W32_OFF["b1_2"]:W32_OFF["b1_2"] + 2]
            b2_sb = c32("b2c"); gp_sb = c32("gammap"); bp_sb = c32("betap")
            be1_sb = c32("be1_2"); bs1_sb = c32("bs1_4"); be2_sb = c32("be2_4")
            be3_sb = c32("be3b"); bs2_sb = c32("bs2b")
            sigb_sb = c32("sigb"); sigs_sb = c32("sigs")
            beA_sb = c32("beA"); beB_sb = c32("beB")
            bsA_sb = c32("bsA"); bsB_sb = c32("bsB")

            eps_sb = consts.tile([128, 1], f32, tag="eps")
            nc.vector.memset(eps_sb[:], 1e-5)

            idf32 = consts.tile([128, 128], f32, tag="idf32")
            make_identity(nc, idf32[:])
            idf16 = consts.tile([128, 128], f16, tag="idf16")
            make_identity(nc, idf16[:])

            h1t = [persist.tile([128, TOK], f16, tag=f"h1t{m}", name=f"h1t{m}") for m in range(2)]
            ht = persist.tile([128, TOK], f32, tag="ht")
            nfraw = persist.tile([128, 8, 128], f32, tag="nfraw")
            nft = persist.tile([128, TOK], f16, tag="nft")
            ab_all = persist.tile([128, 8, 192], f16, tag="ab_all")
            ab = [ab_all[:, g, :] for g in range(8)]
            apbp_all = persist.tile([128, 4, 192], f16, tag="apbp_all")
            apbp = [apbp_all[:, gr, :] for gr in range(4)]
            e2r = [persist.tile([128, N * N], f16, tag=f"e2r{u}", name=f"e2r{u}") for u in range(4)]
            s1r = [persist.tile([128, N * N], f16, tag=f"s1r{gr}", name=f"s1r{gr}") for gr in range(4)]

            # ================= encoder =================
            with tc.tile_pool(name="ps_enc", bufs=2, space="PSUM") as ps_enc:
                for m in range(2):
                    for n2 in range(2):
                        ps = ps_enc.tile([128, 512], f32, tag="enc", name="ps_enc_t")
                        for c in range(4):
                            nc.tensor.matmul(
                                ps[:],
                                lhsT=w1_sb[c][:, m * 128:(m + 1) * 128],
                                rhs=xt_sb[c][:, n2 * 512:(n2 + 1) * 512],
                                start=(c == 0), stop=(c == 3),
                            )
                        nc.scalar.activation(
                            out=h1t[m][:, n2 * 512:(n2 + 1) * 512], in_=ps[:],
                            func=AF.Relu, bias=b1_sb[:, m:m + 1], scale=1.0,
                        )
                for n2 in range(2):
                    ps = ps_enc.tile([128, 512], f32, tag="enc", name="ps_enc_t")
                    for c2 in range(2):
                        nc.tensor.matmul(
                            ps[:], lhsT=w2_sb[c2][:],
                            rhs=h1t[c2][:, n2 * 512:(n2 + 1) * 512],
                            start=(c2 == 0), stop=(c2 == 1),
                        )
                    nc.scalar.activation(
                        out=ht[:, n2 * 512:(n2 + 1) * 512], in_=ps[:],
                        func=AF.Identity, bias=b2_sb[:, 0:1], scale=1.0,
                    )

            # ================= layernorm (token-major) =================
            # sqrt-free: rstd = Newton-rsqrt from 1/x seed (keeps a single
            # ACT table set resident for the whole kernel)
            mu_all = persist.tile([128, 8], f32, tag="mu_all")
            rstd_all = persist.tile([128, 8], f32, tag="rstd_all")
            ve_all = persist.tile([128, 8], f32, tag="ve_all")
            nmr_all = persist.tile([128, 8], f32, tag="nmr_all")
            with tc.tile_pool(name="ps_tr", bufs=1, space="PSUM") as ps_tr, \
                 tc.tile_pool(name="ln_tmp", bufs=4) as ln_tmp:
                pstok = ps_tr.tile([128, 1024], f32, tag="htok", name="pstok")
                for t in range(8):
                    nc.tensor.transpose(pstok[:, t * 128:(t + 1) * 128],
                                        ht[:, t * 128:(t + 1) * 128], idf32[:])
                st6 = ln_tmp.tile([128, 8, 6], f32, tag="st6", name="st6")
                p3 = pstok[:].rearrange("p (t f) -> p t f", t=8)
                for t in range(8):
                    nc.vector.bn_stats(out=st6[:, t, :], in_=p3[:, t, :])
                mv3 = ln_tmp.tile([128, 8, 2], f32, tag="mv3", name="mv3")
                for t in range(8):
                    nc.vector.bn_aggr(out=mv3[:, t, :], in_=st6[:, t, :])
                nc.vector.tensor_copy(mu_all[:], mv3[:, :, 0])
                # rstd = rsqrt(ve + eps), Newton x3 from linear-in-1/x seed
                nc.vector.tensor_scalar_add(out=ve_all[:], in0=mv3[:, :, 1],
                                            scalar1=1e-5)
                rr = ln_tmp.tile([128, 8], f32, tag="rr", name="rr")
                nc.vector.reciprocal(out=rr[:], in_=ve_all[:])
                nc.vector.tensor_scalar(out=rstd_all[:], in0=rr[:],
                                        scalar1=0.345, scalar2=0.53,
                                        op0=OP.mult, op1=OP.add)
                t2 = ln_tmp.tile([128, 8], f32, tag="t2", name="t2")
                for _ in range(3):
                    nc.vector.tensor_mul(t2[:], rstd_all[:], rstd_all[:])
                    nc.vector.tensor_mul(t2[:], t2[:], ve_all[:])
                    nc.vector.tensor_scalar(out=t2[:], in0=t2[:],
                                            scalar1=-0.5, scalar2=1.5,
                                            op0=OP.mult, op1=OP.add)
                    nc.vector.tensor_mul(rstd_all[:], rstd_all[:], t2[:])
                nc.vector.tensor_mul(nmr_all[:], mu_all[:], rstd_all[:])
                nc.vector.tensor_scalar_mul(out=nmr_all[:], in0=nmr_all[:],
                                            scalar1=-1.0)
                for t in range(8):
                    nc.scalar.activation(
                        out=nfraw[:, t, :], in_=pstok[:, t * 128:(t + 1) * 128],
                        func=AF.Identity, bias=nmr_all[:, t:t + 1],
                        scale=rstd_all[:, t:t + 1])
                nc.sync.dma_start(
                    out=nf_d[:].rearrange("(t p) f -> p t f", t=8, p=128),
                    in_=nfraw[:])

                # nfT (feature-major) with gamma/beta applied per-partition
                psn = ps_tr.tile([128, 1024], f32, tag="nft_ps", name="psn")
                for t in range(8):
                    nc.tensor.transpose(psn[:, t * 128:(t + 1) * 128],
                                        nfraw[:, t, :], idf32[:])
                nc.scalar.activation(out=nft[:], in_=psn[:], func=AF.Copy)

            # ================= pair projections =================
            # Fat matmuls with strided token-gather APs: for each partition
            # parity bb, one matmul gathers the i-side (A) columns of all its
            # groups; the j-side (B) is emitted with duplicated columns for
            # the 2x-mode pair build. Bias be1/bs1 is added as bias/2 on both
            # the A and B evacuations.
            with tc.tile_pool(name="ps_proj", bufs=1, space="PSUM") as ps_proj:
                psE = ps_proj.tile([128, 1536], f32, tag="proj_e",
                                   name="psE")
                nftv = nft[:]
                for bb in range(2):
                    # A: [64f, 8 groups x 64 cols]; tokens of b = 2g + bb
                    rhs_a = bass.AP(
                        tensor=nftv.tensor, offset=nftv.offset + bb * N,
                        ap=[list(nftv.ap[0]), [2 * N, 8], [1, N]])
                    nc.tensor.matmul(
                        psE[64 * bb:64 * bb + 64, 0:512], lhsT=we1a_sb[:],
                        rhs=rhs_a, start=True, stop=True,
                        tile_position=(0, 64 * bb))
                    # B dup: [64f, 8 groups x 128 cols] (two 512-col mms)
                    for hf in range(2):
                        rhs_b = bass.AP(
                            tensor=nftv.tensor,
                            offset=nftv.offset + bb * N + hf * 4 * 2 * N,
                            ap=[list(nftv.ap[0]), [2 * N, 4], [1, N], [0, 2]])
                        nc.tensor.matmul(
                            psE[64 * bb:64 * bb + 64,
                                512 + hf * 512:512 + (hf + 1) * 512],
                            lhsT=we1b_sb[:], rhs=rhs_b, start=True, stop=True,
                            tile_position=(0, 64 * bb))
                nc.scalar.activation(
                    out=ab_all[:, :, 0:64],
                    in_=psE[:, 0:512].rearrange("p (g n) -> p g n", g=8),
                    func=AF.Identity, bias=beA_sb[:, 0:1], scale=1.0)
                nc.scalar.activation(
                    out=ab_all[:, :, 64:192],
                    in_=psE[:, 512:1536].rearrange("p (g n) -> p g n", g=8),
                    func=AF.Identity, bias=beB_sb[:, 0:1], scale=1.0)

                psS = ps_proj.tile([128, 1024], f32, tag="proj_s", name="psS")
                for bb in range(4):
                    rhs_a = bass.AP(
                        tensor=nftv.tensor, offset=nftv.offset + bb * N,
                        ap=[list(nftv.ap[0]), [4 * N, 4], [1, N]])
                    nc.tensor.matmul(
                        psS[32 * bb:32 * bb + 32, 0:256], lhsT=ws1a_sb[:],
                        rhs=rhs_a, start=True, stop=True,
                        tile_position=(0, 32 * bb))
                    rhs_b = bass.AP(
                        tensor=nftv.tensor, offset=nftv.offset + bb * N,
                        ap=[list(nftv.ap[0]), [4 * N, 4], [1, N], [0, 2]])
                    nc.tensor.matmul(
                        psS[32 * bb:32 * bb + 32, 512:1024], lhsT=ws1b_sb[:],
                        rhs=rhs_b, start=True, stop=True,
                        tile_position=(0, 32 * bb))
                nc.scalar.activation(
                    out=apbp_all[:, :, 0:64],
                    in_=psS[:, 0:256].rearrange("p (g n) -> p g n", g=4),
                    func=AF.Identity, bias=bsA_sb[:, 0:1], scale=1.0)
                nc.scalar.activation(
                    out=apbp_all[:, :, 64:192],
                    in_=psS[:, 512:1024].rearrange("p (g n) -> p g n", g=4),
                    func=AF.Identity, bias=bsB_sb[:, 0:1], scale=1.0)

            # ================= N^2 builds + edge layer2 =================
            # E-path: per u (2 edge groups): DVE builds -> PE e2 -> ACT evac,
            # with finals-half0 matmuls interleaved (PSUM: e2 4 banks +
            # finals-h0 4 banks). S-path runs at the tail, overlapping
            # E-finals/sigmoid.
            relu_act_counter = [0]

            def build_dve(out3, src):
                out4 = out3.rearrange("p j (i2 ii) -> p j i2 ii", ii=2)
                a4 = src[:, None, 0:64].broadcast_to((128, N, 64)) \
                    .rearrange("p j (i2 ii) -> p j i2 ii", ii=2)
                b4 = src[:, 64:192].rearrange("p (j ii) -> p j ii", ii=2)[
                    :, :, None, :].broadcast_to((128, N, 32, 2))
                k = relu_act_counter[0]
                relu_act_counter[0] += 1
                flat = out3.rearrange("p i j -> p (i j)")
                for h in range(2):
                    nc.vector.tensor_add(out4[:, 32 * h:32 * h + 32],
                                         b4[:, 32 * h:32 * h + 32],
                                         a4[:, 32 * h:32 * h + 32])
                    hf = flat[:, h * 2048:(h + 1) * 2048]
                    if k % 4 == 1:    # every 4th tile relu on ACT
                        nc.scalar.activation(out=hf, in_=hf, func=AF.Relu)
                    else:
                        nc.vector.tensor_scalar_max(out=hf, in0=hf,
                                                    scalar1=0.0)

            adj_t = [None, None]
            str_t = [None, None]
            with tc.tile_pool(name="e1pool", bufs=4) as e1pool, \
                 tc.tile_pool(name="ps_e2", bufs=2, space="PSUM") as ps_e2, \
                 tc.tile_pool(name="ps_finE0", bufs=1, space="PSUM") as ps_finE0, \
                 tc.tile_pool(name="outstage", bufs=1) as outstage:
                finE0 = ps_finE0.tile([128, 2048], f32, tag="fe0", name="finE0")
                for u in range(4):
                    e1t = []
                    for g in (2 * u, 2 * u + 1):
                        e1 = e1pool.tile([128, N, N], f16, tag="e1",
                                         name=f"e1_{g}")
                        build_dve(e1[:], ab[g])
                        e1t.append(e1[:].rearrange("p i j -> p (i j)"))
                    for cp in range(4):
                        ps = ps_e2.tile([128, 1024], f32, tag="e2",
                                        name="ps_e2_t")
                        for hf in range(2):
                            c0 = cp * 1024 + hf * 512
                            nc.tensor.matmul(
                                ps[0:64, hf * 512:(hf + 1) * 512],
                                lhsT=bd2we2_sb[:], rhs=e1t[0][:, c0:c0 + 512],
                                start=True, stop=True, tile_position=(0, 0),
                            )
                            nc.tensor.matmul(
                                ps[64:128, hf * 512:(hf + 1) * 512],
                                lhsT=bd2we2_sb[:], rhs=e1t[1][:, c0:c0 + 512],
                                start=True, stop=True, tile_position=(0, 64),
                            )
                        nc.scalar.activation(
                            out=e2r[u][:, cp * 1024:(cp + 1) * 1024], in_=ps[:],
                            func=AF.Relu, bias=be2_sb[:, 0:1], scale=1.0,
                        )
                    for ch in range(4):     # finals half 0 for this u
                        nc.tensor.matmul(
                            finE0[32 * u:32 * u + 4, ch * 512:(ch + 1) * 512],
                            lhsT=bd4we3_sb[:], rhs=e2r[u][:, ch * 512:
                                                          (ch + 1) * 512],
                            start=True, stop=True, tile_position=(0, 32 * u),
                        )
                    build_dve(s1r[u][:].rearrange("p (i j) -> p i j", i=N),
                              apbp[u])
                adj_t[0] = outstage.tile([128, 2048], f16, tag="adj0",
                                         name="adj_t0")
                nc.scalar.activation(out=adj_t[0][:], in_=finE0[:],
                                     func=AF.Sigmoid, bias=be3_sb[:, 0:1],
                                     scale=1.0)
                nc.sync.dma_start(out=pairs_d[0, :, 0:1024],
                                  in_=adj_t[0][:, 0:1024])
                nc.gpsimd.dma_start(out=pairs_d[0, :, 1024:2048],
                                    in_=adj_t[0][:, 1024:2048])

            # tail: S finals h0 -> E finals h1 -> S finals h1
            with tc.tile_pool(name="ps_fin2", bufs=2, space="PSUM") as ps_fin2, \
                 tc.tile_pool(name="outstage2", bufs=2) as outstage2:
                def head_mms(ps, lhsT, rhs_tiles, c0):
                    for ch in range(4):
                        for k in range(4):
                            nc.tensor.matmul(
                                ps[32 * k:32 * k + 4, ch * 512:(ch + 1) * 512],
                                lhsT=lhsT,
                                rhs=rhs_tiles[k][:, c0 + ch * 512:
                                                 c0 + (ch + 1) * 512],
                                start=True, stop=True,
                                tile_position=(0, 32 * k),
                            )

                s1aps = [s1r[gr][:] for gr in range(4)]
                e2aps = [e2r[u][:] for u in range(4)]
                ps_s0 = ps_fin2.tile([128, 2048], f32, tag="fin",
                                     name="ps_s0")
                head_mms(ps_s0, bd4ws2_sb[:], s1aps, 0)
                st0 = outstage2.tile([128, 2048], f16, tag="o2", name="st0")
                nc.scalar.activation(out=st0[:], in_=ps_s0[:], func=AF.Tanh,
                                     bias=bs2_sb[:, 0:1], scale=1.0)
                nc.sync.dma_start(out=pairs_d[1, :, 0:1024],
                                  in_=st0[:, 0:1024])
                nc.gpsimd.dma_start(out=pairs_d[1, :, 1024:2048],
                                    in_=st0[:, 1024:2048])

                ps_e1 = ps_fin2.tile([128, 2048], f32, tag="fin",
                                     name="ps_e1f")
                head_mms(ps_e1, bd4we3_sb[:], e2aps, 2048)
                at1 = outstage2.tile([128, 2048], f16, tag="o2", name="at1")
                nc.scalar.activation(out=at1[:], in_=ps_e1[:], func=AF.Sigmoid,
                                     bias=be3_sb[:, 0:1], scale=1.0)
                nc.sync.dma_start(out=pairs_d[0, :, 2048:3072],
                                  in_=at1[:, 0:1024])
                nc.gpsimd.dma_start(out=pairs_d[0, :, 3072:4096],
                                    in_=at1[:, 1024:2048])

                ps_s1 = ps_fin2.tile([128, 2048], f32, tag="fin",
                                     name="ps_s1f")
                head_mms(ps_s1, bd4ws2_sb[:], s1aps, 2048)
                st1 = outstage2.tile([128, 2048], f16, tag="o2", name="st1")
                nc.scalar.activation(out=st1[:], in_=ps_s1[:], func=AF.Tanh,
                                     bias=bs2_sb[:, 0:1], scale=1.0)
                nc.sync.dma_start(out=pairs_d[1, :, 2048:3072],
                                  in_=st1[:, 0:1024])
                nc.gpsimd.dma_start(out=pairs_d[1, :, 3072:4096],
                                    in_=st1[:, 1024:2048])

    _split_multiwait(nc)
    return nc


def _get_program():
    if "nc" not in _PROGRAM_CACHE:
        _PROGRAM_CACHE["nc"] = _build_program()
    return _PROGRAM_CACHE["nc"]


# ---------------------------------------------------------------------------
# host wrapper
# ---------------------------------------------------------------------------

def kernel(state_sequence, W1, b1, W2, b2, gamma, beta,
           We1, be1, We2, be2, We3, be3, Ws1, bs1, Ws2, bs2):
    global LAST_RESULTS
    from concourse.bass_utils import run_bass_kernel_spmd

    state_sequence = np.asarray(state_sequence, dtype=np.float32)
    W1 = np.asarray(W1, np.float32); b1 = np.asarray(b1, np.float32)
    W2 = np.asarray(W2, np.float32); b2 = np.asarray(b2, np.float32)
    gamma = np.asarray(gamma, np.float32); beta = np.asarray(beta, np.float32)
    We1 = np.asarray(We1, np.float32); be1 = np.asarray(be1, np.float32)
    We2 = np.asarray(We2, np.float32); be2 = np.asarray(be2, np.float32)
    We3 = np.asarray(We3, np.float32); be3 = np.asarray(be3, np.float32)
    Ws1 = np.asarray(Ws1, np.float32); bs1 = np.asarray(bs1, np.float32)
    Ws2 = np.asarray(Ws2, np.float32); bs2 = np.asarray(bs2, np.float32)

    idx = _node_indices()
    nodes = state_sequence[:, idx]                      # [B, N, D]

    def f16(a):
        return np.ascontiguousarray(a.astype(np.float16))

    def f32c(a):
        return np.ascontiguousarray(a.astype(np.float32))

    def blockdiag(w, k):
        m, n = w.shape
        out = np.zeros((m * k, n * k), np.float32)
        for i in range(k):
            out[i * m:(i + 1) * m, i * n:(i + 1) * n] = w
        return out

    parts16 = {
        "w1_0": W1[0:128], "w1_1": W1[128:256], "w1_2": W1[256:384],
        "w1_3": W1[384:512],
        "w2_0": W2[0:128], "w2_1": W2[128:256],
        "we1a": gamma[:, None] * We1[:128, :],
        "we1b": gamma[:, None] * We1[128:, :],
        "ws1a": gamma[:, None] * Ws1[:128, :],
        "ws1b": gamma[:, None] * Ws1[128:, :],
        "bd2we2": blockdiag(We2, 2),
        "bd4we3": blockdiag(We3, 4),
        "bd4ws2": blockdiag(Ws2, 4),
    }
    wb16 = np.zeros((128, W16_COLS), np.float16)
    for k, (o, w) in W16_OFF.items():
        wb16[:, o:o + w] = parts16[k].astype(np.float16)
    parts32 = {
        "b1_2": b1.reshape(2, 128).T, "b2c": b2.reshape(128, 1),
        "gammap": gamma.reshape(128, 1), "betap": beta.reshape(128, 1),
        "be1_2": np.tile(be1, 2).reshape(128, 1),
        "bs1_4": np.tile(bs1, 4).reshape(128, 1),
        "be2_4": np.tile(be2, 4).reshape(128, 1),
        "be3b": np.full((128, 1), float(be3[0])),
        "bs2b": np.full((128, 1), float(bs2[0])),
        # per-partition sigmoid affine: edge rows (q%32<4): sigmoid(x+be3);
        # strength rows (4<=q%32<8): sigmoid(2x+2*bs2) -> host maps 2y-1=tanh
        "sigb": np.array([[float(be3[0]) if (q % 32) < 4 else
                           2.0 * float(bs2[0])] for q in range(128)],
                         np.float64),
        "sigs": np.array([[1.0 if (q % 32) < 4 else 2.0] for q in range(128)],
                         np.float64),
        "beA": np.tile(0.5 * be1 + We1[:128].T @ beta, 2).reshape(128, 1),
        "beB": np.tile(0.5 * be1 + We1[128:].T @ beta, 2).reshape(128, 1),
        "bsA": np.tile(0.5 * bs1 + Ws1[:128].T @ beta, 4).reshape(128, 1),
        "bsB": np.tile(0.5 * bs1 + Ws1[128:].T @ beta, 4).reshape(128, 1),
    }
    wb32 = np.zeros((128, W32_COLS), np.float32)
    for k, v in parts32.items():
        o = W32_OFF[k]
        wb32[:, o:o + v.shape[1]] = v
    shared = {"wb16": wb16, "wb32": f32c(wb32)}

    in_maps = []
    for c in range(NCORES):
        xt = f16(nodes[c * BSH:(c + 1) * BSH].reshape(TOK, D).T)
        in_maps.append({"xt": xt, **shared})

    nc = _get_program()
    res = run_bass_kernel_spmd(nc, in_maps, core_ids=list(range(NCORES)))
    LAST_RESULTS = res

    nf = np.empty((B, N, DM), np.float32)
    adj = np.empty((B, N, N), np.float32)
    stg = np.empty((B, N, N), np.float32)
    mask = (1.0 - np.eye(N, dtype=np.float32))
    rows_adj = np.array([32 * (b // 4) + b % 4 for b in range(BSH)])
    rows_str = rows_adj + 4
    for c in range(NCORES):
        r = res.results[c]
        nf_c = r["nf_raw"].reshape(BSH, N, DM) * gamma + beta
        nf[c * BSH:(c + 1) * BSH] = nf_c
        pr = r["pairs"].astype(np.float32)
        adj[c * BSH:(c + 1) * BSH] = \
            pr[0][rows_adj].reshape(BSH, N, N).transpose(0, 2, 1) * mask
        stg[c * BSH:(c + 1) * BSH] = \
            pr[1][rows_adj].reshape(BSH, N, N).transpose(0, 2, 1) * mask
    return nf, adj, stg


# revision 31
# speedup vs baseline: 1.2232x; 1.0167x over previous
"""CausalGraphBuilder Trainium2 kernel.

Full inputs -> shard batch (128) over 8 NeuronCores (16 each) -> Bass kernel
(encoder MLP + LayerNorm + N^2 pair-MLP edge/strength heads) -> gather.

Self-contained: hardcodes B,S,D,N = 128,1024,512,64 and the sharding.
"""

import numpy as np

B, S, D, N = 128, 1024, 512, 64
DM = D // 4          # 128 node-feature dim after encoder
NCORES = 8
BSH = B // NCORES    # 16 batch per core
TOK = BSH * N        # 1024 tokens per core

# jnp.linspace(0, S-1, N).astype(int32) as evaluated by the reference in this
# environment (device cast rounds); recomputed at runtime when jax is available.
_FALLBACK_IDX = [0, 16, 32, 49, 65, 81, 97, 114, 130, 146, 162, 179, 195, 211,
                 227, 244, 260, 276, 292, 309, 325, 341, 357, 373, 390, 406,
                 422, 438, 455, 471, 487, 503, 520, 536, 552, 568, 585, 601,
                 617, 633, 650, 666, 682, 698, 714, 731, 747, 763, 779, 796,
                 812, 828, 844, 861, 877, 893, 909, 926, 942, 958, 974, 991,
                 1007, 1023]


def _node_indices():
    try:
        import jax.numpy as jnp
        idx = np.asarray(jnp.linspace(0.0, float(S - 1), N).astype(jnp.int32))
        if idx.shape == (N,):
            return idx.astype(np.int64)
    except Exception:
        pass
    return np.array(_FALLBACK_IDX, dtype=np.int64)


# ---------------------------------------------------------------------------
# device program
# ---------------------------------------------------------------------------

_PROGRAM_CACHE = {}
LAST_RESULTS = None  # BassKernelResults of the most recent run (for test.py)

# engine assignment knobs (tuned against the profile)
PE_E_TILES = ()             # E-groups built on PE (identity-matmul 2-pass)
PE_S_TILES = (1, 3)     # S-groups built on PE
EVAC_DVE_E = ()               # PE-built E tiles evacuated by DVE instead of ACT
EVAC_DVE_S = ()
GP_RELU = False               # DVE-built tiles: relu on GPSIMD (slow ucode!)
E2RELU_DVE = ()               # e2 cp-chunks (0..3) whose relu-evac runs on DVE



# packed-constant blob layouts (columns)
_W16_SECTIONS = [
    ("w1_0", 256), ("w1_1", 256), ("w1_2", 256), ("w1_3", 256),
    ("w2_0", 128), ("w2_1", 128),
    ("we1a", 64), ("we1b", 64), ("ws1a", 32), ("ws1b", 32),
    ("bd2we2", 64), ("bd4we3", 4), ("bd4ws2", 4),
]
W16_OFF = {}
_o = 0
for _k, _w in _W16_SECTIONS:
    W16_OFF[_k] = (_o, _w)
    _o += _w
W16_COLS = _o
_W32_SECTIONS = [("b1_2", 2), ("b2c", 1), ("gammap", 1), ("betap", 1),
                 ("be1_2", 1), ("bs1_4", 1), ("be2_4", 1), ("be3b", 1),
                 ("bs2b", 1), ("sigb", 1), ("sigs", 1), ("beA", 1),
                 ("beB", 1), ("bsA", 1), ("bsB", 1)]
W32_OFF = {}
_o = 0
for _k, _w in _W32_SECTIONS:
    W32_OFF[_k] = _o
    _o += _w
W32_COLS = _o

def _patched_tile_context(nc):
    """TileContext whose tail drain never carries >1 sync wait (this walrus
    build rejects multi-wait CTRL instructions)."""
    import concourse.mybir as mybir
    import concourse.tile as tile
    from concourse.vector_clock import ScopedClock

    class TileContextP(tile.TileContext):
        def _drain_and_barrier(self, tick_clock, wait_clock):
            drain_inst = self.nc.sync.drain()
            wait_clock.add_sem_waits(
                drain_inst.ins, ScopedClock({None: tick_clock.global_clock})
            )
            si = drain_inst.ins.sync_info
            if si is not None and si.on_wait is not None and len(si.on_wait) > 1:
                waits = list(si.on_wait)
                si.on_wait = waits[:1]
                for w in waits[1:]:
                    extra = self.nc.sync.drain()
                    extra.ins.sync_info = mybir.SyncInfo(on_wait=[w], on_update=[])
            self.nc.all_engine_barrier()
            popped = self.nc._tile_sem_poison_stack.pop()
            assert popped is self._sem_poison
            self.nc.clear_and_free_semaphores(list(self.sems.allocated().values()))

    return TileContextP(nc)


def _split_multiwait(nc):
    """This walrus build accepts at most ONE sync wait per instruction; hoist
    extra waits into single-wait NoOps on the same engine just before."""
    import concourse.mybir as mybir

    n_split = 0
    for f in nc.m.functions:
        for bb in f.blocks:
            insts = list(bb.instructions)
            out = []
            for ins in insts:
                si = ins.sync_info
                if si is not None and si.on_wait is not None and len(si.on_wait) > 1:
                    waits = list(si.on_wait)
                    for w in waits[:-1]:
                        nop = mybir.InstNoOp(
                            name=f"{ins.name}-w{n_split}",
                            engine=ins.engine,
                            bass_nofuse=True,
                            sync_info=mybir.SyncInfo(on_wait=[w], on_update=[]),
                        )
                        out.append(nop)
                        n_split += 1
                    si.on_wait = waits[-1:]
                out.append(ins)
            if n_split:
                bb.instructions = out
    return n_split


def _build_program():
    import concourse.bass as bass
    import concourse.mybir as mybir
    from concourse.masks import make_identity

    f16 = mybir.dt.float16
    f32 = mybir.dt.float32
    AF = mybir.ActivationFunctionType
    OP = mybir.AluOpType

    nc = bass.Bass()

    # ---- DRAM I/O ----
    xt_d = nc.dram_tensor("xt", [D, TOK], f16, kind="ExternalInput")
    wb16_d = nc.dram_tensor("wb16", [128, W16_COLS], f16, kind="ExternalInput")
    wb32_d = nc.dram_tensor("wb32", [128, W32_COLS], f32, kind="ExternalInput")

    nf_d = nc.dram_tensor("nf_raw", [TOK, DM], f32, kind="ExternalOutput")
    pairs_d = nc.dram_tensor("pairs", [2, 128, N * N], f16, kind="ExternalOutput")

    with _patched_tile_context(nc) as tc:
        from contextlib import ExitStack

        with ExitStack() as ctx:
            consts = ctx.enter_context(tc.tile_pool(name="consts", bufs=1))
            persist = ctx.enter_context(tc.tile_pool(name="persist", bufs=1))

            # ---- load constants ----
            def load(pool, name, dram, shape, dtype):
                t = pool.tile(shape, dtype, tag=name, name=name)
                nc.sync.dma_start(out=t[:], in_=dram[:])
                return t

            wb32 = load(consts, "wb32", wb32_d, [128, W32_COLS], f32)
            wb16 = load(consts, "wb16", wb16_d, [128, W16_COLS], f16)
            dummy_sig = consts.tile([128, 1], f32, tag="dsig", name="dummy_sig")
            nc.scalar.activation(out=dummy_sig[:], in_=wb32[:, 0:1],
                                 func=AF.Sigmoid)
            xt_sb4 = consts.tile([128, 4, TOK], f16, tag="xt", name="xt_sb4")
            for c in range(4):
                eng = nc.sync if c % 2 == 0 else nc.gpsimd
                eng.dma_start(
                    out=xt_sb4[:, c, :], in_=xt_d[c * 128:(c + 1) * 128, :])
            xt_sb = [xt_sb4[:, c, :] for c in range(4)]

            def c16(key):
                o, w = W16_OFF[key]
                return wb16[:, o:o + w]

            def c32(key):
                o = W32_OFF[key]
                return wb32[:, o:o + 1]

            w1_sb = [c16(f"w1_{c}") for c in range(4)]
            w2_sb = [c16(f"w2_{c}") for c in range(2)]
            we1a_sb = c16("we1a"); we1b_sb = c16("we1b")
            ws1a_sb = c16("ws1a"); ws1b_sb = c16("ws1b")
            bd2we2_sb = c16("bd2we2")
            bd4we3_sb = c16("bd4we3"); bd4ws2_sb = c16("bd4ws2")
            b1_sb = wb32[:, W32_OFF["b1_2"]:W32_OFF["b1_2"] + 2]
            b2_sb = c32("b2c"); gp_sb = c32("gammap"); bp_sb = c32("betap")
            be1_sb = c32("be1_2"); bs1_sb = c32("bs1_4"); be2_sb = c32("be2_4")
            be3_sb = c32("be3b"); bs2_sb = c32("bs2b")
            sigb_sb = c32("sigb"); sigs_sb = c32("sigs")
            beA_sb = c32("beA"); beB_sb = c32("beB")
            bsA_sb = c32("bsA"); bsB_sb = c32("bsB")

            eps_sb = consts.tile([128, 1], f32, tag="eps")
            nc.vector.memset(eps_sb[:], 1e-5)

            idf32 = consts.tile([128, 128], f32, tag="idf32")
            make_identity(nc, idf32[:])
            idf16 = consts.tile([128, 128], f16, tag="idf16")
            make_identity(nc, idf16[:])

            h1t = [persist.tile([128, TOK], f16, tag=f"h1t{m}", name=f"h1t{m}") for m in range(2)]
            ht = persist.tile([128, TOK], f32, tag="ht")
            nfraw = persist.tile([128, 8, 128], f32, tag="nfraw")
            nft = persist.tile([128, TOK], f16, tag="nft")
            ab_all = persist.tile([128, 8, 192], f16, tag="ab_all")
            ab = [ab_all[:, g, :] for g in range(8)]
            apbp_all = persist.tile([128, 4, 192], f16, tag="apbp_all")
            apbp = [apbp_all[:, gr, :] for gr in range(4)]
            e2r = [persist.tile([128, N * N], f16, tag=f"e2r{u}", name=f"e2r{u}") for u in range(4)]
            s1r = [persist.tile([128, N * N], f16, tag=f"s1r{gr}", name=f"s1r{gr}") for gr in range(4)]

            # ================= encoder =================
            with tc.tile_pool(name="ps_enc", bufs=2, space="PSUM") as ps_enc:
                for m in range(2):
                    for n2 in range(2):
                        ps = ps_enc.tile([128, 512], f32, tag="enc", name="ps_enc_t")
                        for c in range(4):
                            nc.tensor.matmul(
                                ps[:],
                                lhsT=w1_sb[c][:, m * 128:(m + 1) * 128],
                                rhs=xt_sb[c][:, n2 * 512:(n2 + 1) * 512],
                                start=(c == 0), stop=(c == 3),
                            )
                        nc.scalar.activation(
                            out=h1t[m][:, n2 * 512:(n2 + 1) * 512], in_=ps[:],
                            func=AF.Relu, bias=b1_sb[:, m:m + 1], scale=1.0,
                        )
                for n2 in range(2):
                    ps = ps_enc.tile([128, 512], f32, tag="enc", name="ps_enc_t")
                    for c2 in range(2):
                        nc.tensor.matmul(
                            ps[:], lhsT=w2_sb[c2][:],
                            rhs=h1t[c2][:, n2 * 512:(n2 + 1) * 512],
                            start=(c2 == 0), stop=(c2 == 1),
                        )
                    nc.scalar.activation(
                        out=ht[:, n2 * 512:(n2 + 1) * 512], in_=ps[:],
                        func=AF.Identity, bias=b2_sb[:, 0:1], scale=1.0,
                    )

            # ================= layernorm (token-major) =================
            # sqrt-free: rstd = Newton-rsqrt from 1/x seed (keeps a single
            # ACT table set resident for the whole kernel)
            mu_all = persist.tile([128, 8], f32, tag="mu_all")
            rstd_all = persist.tile([128, 8], f32, tag="rstd_all")
            ve_all = persist.tile([128, 8], f32, tag="ve_all")
            nmr_all = persist.tile([128, 8], f32, tag="nmr_all")
            with tc.tile_pool(name="ps_tr", bufs=1, space="PSUM") as ps_tr, \
                 tc.tile_pool(name="ln_tmp", bufs=4) as ln_tmp:
                pstok = ps_tr.tile([128, 1024], f32, tag="htok", name="pstok")
                for t in range(8):
                    nc.tensor.transpose(pstok[:, t * 128:(t + 1) * 128],
                                        ht[:, t * 128:(t + 1) * 128], idf32[:])
                st6 = ln_tmp.tile([128, 8, 6], f32, tag="st6", name="st6")
                p3 = pstok[:].rearrange("p (t f) -> p t f", t=8)
                for t in range(8):
                    nc.vector.bn_stats(out=st6[:, t, :], in_=p3[:, t, :])
                mv3 = ln_tmp.tile([128, 8, 2], f32, tag="mv3", name="mv3")
                for t in range(8):
                    nc.vector.bn_aggr(out=mv3[:, t, :], in_=st6[:, t, :])
                nc.vector.tensor_copy(mu_all[:], mv3[:, :, 0])
                # rstd = rsqrt(ve + eps), Newton x3 from linear-in-1/x seed
                nc.vector.tensor_scalar_add(out=ve_all[:], in0=mv3[:, :, 1],
                                            scalar1=1e-5)
                rr = ln_tmp.tile([128, 8], f32, tag="rr", name="rr")
                nc.vector.reciprocal(out=rr[:], in_=ve_all[:])
                nc.vector.tensor_scalar(out=rstd_all[:], in0=rr[:],
                                        scalar1=0.345, scalar2=0.53,
                                        op0=OP.mult, op1=OP.add)
                t2 = ln_tmp.tile([128, 8], f32, tag="t2", name="t2")
                for _ in range(3):
                    nc.vector.tensor_mul(t2[:], rstd_all[:], rstd_all[:])
                    nc.vector.tensor_mul(t2[:], t2[:], ve_all[:])
                    nc.vector.tensor_scalar(out=t2[:], in0=t2[:],
                                            scalar1=-0.5, scalar2=1.5,
                                            op0=OP.mult, op1=OP.add)
                    nc.vector.tensor_mul(rstd_all[:], rstd_all[:], t2[:])
                nc.vector.tensor_mul(nmr_all[:], mu_all[:], rstd_all[:])
                nc.vector.tensor_scalar_mul(out=nmr_all[:], in0=nmr_all[:],
                                            scalar1=-1.0)
                for t in range(8):
                    nc.scalar.activation(
                        out=nfraw[:, t, :], in_=pstok[:, t * 128:(t + 1) * 128],
                        func=AF.Identity, bias=nmr_all[:, t:t + 1],
                        scale=rstd_all[:, t:t + 1])
                nc.sync.dma_start(
                    out=nf_d[:].rearrange("(t p) f -> p t f", t=8, p=128),
                    in_=nfraw[:])

                # nfT (feature-major) with gamma/beta applied per-partition
                psn = ps_tr.tile([128, 1024], f32, tag="nft_ps", name="psn")
                for t in range(8):
                    nc.tensor.transpose(psn[:, t * 128:(t + 1) * 128],
                                        nfraw[:, t, :], idf32[:])
                nc.scalar.activation(out=nft[:], in_=psn[:], func=AF.Copy)

            # ================= pair projections =================
            # Fat matmuls with strided token-gather APs: for each partition
            # parity bb, one matmul gathers the i-side (A) columns of all its
            # groups; the j-side (B) is emitted with duplicated columns for
            # the 2x-mode pair build. Bias be1/bs1 is added as bias/2 on both
            # the A and B evacuations.
            with tc.tile_pool(name="ps_proj", bufs=1, space="PSUM") as ps_proj:
                psE = ps_proj.tile([128, 1536], f32, tag="proj_e",
                                   name="psE")
                nftv = nft[:]
                for bb in range(2):
                    # A: [64f, 8 groups x 64 cols]; tokens of b = 2g + bb
                    rhs_a = bass.AP(
                        tensor=nftv.tensor, offset=nftv.offset + bb * N,
                        ap=[list(nftv.ap[0]), [2 * N, 8], [1, N]])
                    nc.tensor.matmul(
                        psE[64 * bb:64 * bb + 64, 0:512], lhsT=we1a_sb[:],
                        rhs=rhs_a, start=True, stop=True,
                        tile_position=(0, 64 * bb))
                    # B dup: [64f, 8 groups x 128 cols] (two 512-col mms)
                    for hf in range(2):
                        rhs_b = bass.AP(
                            tensor=nftv.tensor,
                            offset=nftv.offset + bb * N + hf * 4 * 2 * N,
                            ap=[list(nftv.ap[0]), [2 * N, 4], [1, N], [0, 2]])
                        nc.tensor.matmul(
                            psE[64 * bb:64 * bb + 64,
                                512 + hf * 512:512 + (hf + 1) * 512],
                            lhsT=we1b_sb[:], rhs=rhs_b, start=True, stop=True,
                            tile_position=(0, 64 * bb))
                nc.scalar.activation(
                    out=ab_all[:, :, 0:64],
                    in_=psE[:, 0:512].rearrange("p (g n) -> p g n", g=8),
                    func=AF.Identity, bias=beA_sb[:, 0:1], scale=1.0)
                nc.scalar.activation(
                    out=ab_all[:, :, 64:192],
                    in_=psE[:, 512:1536].rearrange("p (g n) -> p g n", g=8),
                    func=AF.Identity, bias=beB_sb[:, 0:1], scale=1.0)

                psS = ps_proj.tile([128, 1024], f32, tag="proj_s", name="psS")
                for bb in range(4):
                    rhs_a = bass.AP(
                        tensor=nftv.tensor, offset=nftv.offset + bb * N,
                        ap=[list(nftv.ap[0]), [4 * N, 4], [1, N]])
                    nc.tensor.matmul(
                        psS[32 * bb:32 * bb + 32, 0:256], lhsT=ws1a_sb[:],
                        rhs=rhs_a, start=True, stop=True,
                        tile_position=(0, 32 * bb))
                    rhs_b = bass.AP(
                        tensor=nftv.tensor, offset=nftv.offset + bb * N,
                        ap=[list(nftv.ap[0]), [4 * N, 4], [1, N], [0, 2]])
                    nc.tensor.matmul(
                        psS[32 * bb:32 * bb + 32, 512:1024], lhsT=ws1b_sb[:],
                        rhs=rhs_b, start=True, stop=True,
                        tile_position=(0, 32 * bb))
                nc.scalar.activation(
                    out=apbp_all[:, :, 0:64],
                    in_=psS[:, 0:256].rearrange("p (g n) -> p g n", g=4),
                    func=AF.Identity, bias=bsA_sb[:, 0:1], scale=1.0)
                nc.scalar.activation(
                    out=apbp_all[:, :, 64:192],
                    in_=psS[:, 512:1024].rearrange("p (g n) -> p g n", g=4),
                    func=AF.Identity, bias=bsB_sb[:, 0:1], scale=1.0)

            # ================= N^2 builds + edge layer2 =================
            # E-path: per u (2 edge groups): DVE builds -> PE e2 -> ACT evac,
            # with finals-half0 matmuls interleaved (PSUM: e2 4 banks +
            # finals-h0 4 banks). S-path runs at the tail, overlapping
            # E-finals/sigmoid.
            relu_act_counter = [0]

            def build_dve(out3, src):
                out4 = out3.rearrange("p j (i2 ii) -> p j i2 ii", ii=2)
                a4 = src[:, None, 0:64].broadcast_to((128, N, 64)) \
                    .rearrange("p j (i2 ii) -> p j i2 ii", ii=2)
                b4 = src[:, 64:192].rearrange("p (j ii) -> p j ii", ii=2)[
                    :, :, None, :].broadcast_to((128, N, 32, 2))
                k = relu_act_counter[0]
                relu_act_counter[0] += 1
                flat = out3.rearrange("p i j -> p (i j)")
                for h in range(2):
                    nc.vector.tensor_add(out4[:, 32 * h:32 * h + 32],
                                         b4[:, 32 * h:32 * h + 32],
                                         a4[:, 32 * h:32 * h + 32])
                    hf = flat[:, h * 2048:(h + 1) * 2048]
                    if k % 4 == 1:    # every 4th tile relu on ACT
                        nc.scalar.activation(out=hf, in_=hf, func=AF.Relu)
                    else:
                        nc.vector.tensor_scalar_max(out=hf, in0=hf,
                                                    scalar1=0.0)

            adj_t = [None, None]
            str_t = [None, None]
            with tc.tile_pool(name="e1pool", bufs=4) as e1pool, \
                 tc.tile_pool(name="ps_e2", bufs=2, space="PSUM") as ps_e2, \
                 tc.tile_pool(name="ps_finE0", bufs=1, space="PSUM") as ps_finE0, \
                 tc.tile_pool(name="outstage", bufs=1) as outstage:
                finE0 = ps_finE0.tile([128, 2048], f32, tag="fe0", name="finE0")
                for u in range(4):
                    e1t = []
                    for g in (2 * u, 2 * u + 1):
                        e1 = e1pool.tile([128, N, N], f16, tag="e1",
                                         name=f"e1_{g}")
                        build_dve(e1[:], ab[g])
                        e1t.append(e1[:].rearrange("p i j -> p (i j)"))
                    for cp in range(4):
                        ps = ps_e2.tile([128, 1024], f32, tag="e2",
                                        name="ps_e2_t")
                        for hf in range(2):
                            c0 = cp * 1024 + hf * 512
                            nc.tensor.matmul(
                                ps[0:64, hf * 512:(hf + 1) * 512],
                                lhsT=bd2we2_sb[:], rhs=e1t[0][:, c0:c0 + 512],
                                start=True, stop=True, tile_position=(0, 0),
                            )
                            nc.tensor.matmul(
                                ps[64:128, hf * 512:(hf + 1) * 512],
                                lhsT=bd2we2_sb[:], rhs=e1t[1][:, c0:c0 + 512],
                                start=True, stop=True, tile_position=(0, 64),
                            )
                        nc.scalar.activation(
                            out=e2r[u][:, cp * 1024:(cp + 1) * 1024], in_=ps[:],
                            func=AF.Relu, bias=be2_sb[:, 0:1], scale=1.0,
                        )
                    for ch in range(4):     # finals half 0 for this u
                        nc.tensor.matmul(
                            finE0[32 * u:32 * u + 4, ch * 512:(ch + 1) * 512],
                            lhsT=bd4we3_sb[:], rhs=e2r[u][:, ch * 512:
                                                          (ch + 1) * 512],
                            start=True, stop=True, tile_position=(0, 32 * u),
                        )
                    build_dve(s1r[u][:].rearrange("p (i j) -> p i j", i=N),
                              apbp[u])
                adj_t[0] = outstage.tile([128, 2048], f16, tag="adj0",
                                         name="adj_t0")
                nc.scalar.activation(out=adj_t[0][:], in_=finE0[:],
                                     func=AF.Sigmoid, bias=be3_sb[:, 0:1],
                                     scale=1.0)
                nc.sync.dma_start(out=pairs_d[0, :, 0:1024],
                                  in_=adj_t[0][:, 0:1024])
                nc.gpsimd.dma_start(out=pairs_d[0, :, 1024:2048],
                                    in_=adj_t[0][:, 1024:2048])

            # tail: S finals h0 -> E finals h1 -> S finals h1
            with tc.tile_pool(name="ps_fin2", bufs=2, space="PSUM") as ps_fin2, \
                 tc.tile_pool(name="outstage2", bufs=2) as outstage2:
                def head_mms(ps, lhsT, rhs_tiles, c0):
                    for ch in range(4):
                        for k in range(4):
                            nc.tensor.matmul(
                                ps[32 * k:32 * k + 4, ch * 512:(ch + 1) * 512],
                                lhsT=lhsT,
                                rhs=rhs_tiles[k][:, c0 + ch * 512:
                                                 c0 + (ch + 1) * 512],
                                start=True, stop=True,
                                tile_position=(0, 32 * k),
                            )

                s1aps = [s1r[gr][:] for gr in range(4)]
                e2aps = [e2r[u][:] for u in range(4)]
                ps_s0 = ps_fin2.tile([128, 2048], f32, tag="fin",
                                     name="ps_s0")
                head_mms(ps_s0, bd4ws2_sb[:], s1aps, 0)
                st0 = outstage2.tile([128, 2048], f16, tag="o2", name="st0")
                nc.scalar.activation(out=st0[:], in_=ps_s0[:], func=AF.Tanh,
                                     bias=bs2_sb[:, 0:1], scale=1.0)
                nc.scalar.dma_start(out=pairs_d[1, :, 0:1024],
                                    in_=st0[:, 0:1024])
                nc.gpsimd.dma_start(out=pairs_d[1, :, 1024:2048],
                                    in_=st0[:, 1024:2048])

                ps_e1 = ps_fin2.tile([128, 2048], f32, tag="fin",
                                     name="ps_e1f")
                head_mms(ps_e1, bd4we3_sb[:], e2aps, 2048)
                at1 = outstage2.tile([128, 2048], f16, tag="o2", name="at1")
                nc.scalar.activation(out=at1[:], in_=ps_e1[:], func=AF.Sigmoid,
                                     bias=be3_sb[:, 0:1], scale=1.0)
                nc.sync.dma_start(out=pairs_d[0, :, 2048:3072],
                                  in_=at1[:, 0:1024])
                nc.gpsimd.dma_start(out=pairs_d[0, :, 3072:4096],
                                    in_=at1[:, 1024:2048])

                ps_s1 = ps_fin2.tile([128, 2048], f32, tag="fin",
                                     name="ps_s1f")
                head_mms(ps_s1, bd4ws2_sb[:], s1aps, 2048)
                st1 = outstage2.tile([128, 2048], f16, tag="o2", name="st1")
                nc.scalar.activation(out=st1[:], in_=ps_s1[:], func=AF.Tanh,
                                     bias=bs2_sb[:, 0:1], scale=1.0)
                nc.scalar.dma_start(out=pairs_d[1, :, 2048:3072],
                                    in_=st1[:, 0:1024])
                nc.gpsimd.dma_start(out=pairs_d[1, :, 3072:4096],
                                    in_=st1[:, 1024:2048])

    _split_multiwait(nc)
    return nc


def _get_program():
    if "nc" not in _PROGRAM_CACHE:
        _PROGRAM_CACHE["nc"] = _build_program()
    return _PROGRAM_CACHE["nc"]


# ---------------------------------------------------------------------------
# host wrapper
# ---------------------------------------------------------------------------

def kernel(state_sequence, W1, b1, W2, b2, gamma, beta,
           We1, be1, We2, be2, We3, be3, Ws1, bs1, Ws2, bs2):
    global LAST_RESULTS
    from concourse.bass_utils import run_bass_kernel_spmd

    state_sequence = np.asarray(state_sequence, dtype=np.float32)
    W1 = np.asarray(W1, np.float32); b1 = np.asarray(b1, np.float32)
    W2 = np.asarray(W2, np.float32); b2 = np.asarray(b2, np.float32)
    gamma = np.asarray(gamma, np.float32); beta = np.asarray(beta, np.float32)
    We1 = np.asarray(We1, np.float32); be1 = np.asarray(be1, np.float32)
    We2 = np.asarray(We2, np.float32); be2 = np.asarray(be2, np.float32)
    We3 = np.asarray(We3, np.float32); be3 = np.asarray(be3, np.float32)
    Ws1 = np.asarray(Ws1, np.float32); bs1 = np.asarray(bs1, np.float32)
    Ws2 = np.asarray(Ws2, np.float32); bs2 = np.asarray(bs2, np.float32)

    idx = _node_indices()
    nodes = state_sequence[:, idx]                      # [B, N, D]

    def f16(a):
        return np.ascontiguousarray(a.astype(np.float16))

    def f32c(a):
        return np.ascontiguousarray(a.astype(np.float32))

    def blockdiag(w, k):
        m, n = w.shape
        out = np.zeros((m * k, n * k), np.float32)
        for i in range(k):
            out[i * m:(i + 1) * m, i * n:(i + 1) * n] = w
        return out

    parts16 = {
        "w1_0": W1[0:128], "w1_1": W1[128:256], "w1_2": W1[256:384],
        "w1_3": W1[384:512],
        "w2_0": W2[0:128], "w2_1": W2[128:256],
        "we1a": gamma[:, None] * We1[:128, :],
        "we1b": gamma[:, None] * We1[128:, :],
        "ws1a": gamma[:, None] * Ws1[:128, :],
        "ws1b": gamma[:, None] * Ws1[128:, :],
        "bd2we2": blockdiag(We2, 2),
        "bd4we3": blockdiag(We3, 4),
        "bd4ws2": blockdiag(Ws2, 4),
    }
    wb16 = np.zeros((128, W16_COLS), np.float16)
    for k, (o, w) in W16_OFF.items():
        wb16[:, o:o + w] = parts16[k].astype(np.float16)
    parts32 = {
        "b1_2": b1.reshape(2, 128).T, "b2c": b2.reshape(128, 1),
        "gammap": gamma.reshape(128, 1), "betap": beta.reshape(128, 1),
        "be1_2": np.tile(be1, 2).reshape(128, 1),
        "bs1_4": np.tile(bs1, 4).reshape(128, 1),
        "be2_4": np.tile(be2, 4).reshape(128, 1),
        "be3b": np.full((128, 1), float(be3[0])),
        "bs2b": np.full((128, 1), float(bs2[0])),
        # per-partition sigmoid affine: edge rows (q%32<4): sigmoid(x+be3);
        # strength rows (4<=q%32<8): sigmoid(2x+2*bs2) -> host maps 2y-1=tanh
        "sigb": np.array([[float(be3[0]) if (q % 32) < 4 else
                           2.0 * float(bs2[0])] for q in range(128)],
                         np.float64),
        "sigs": np.array([[1.0 if (q % 32) < 4 else 2.0] for q in range(128)],
                         np.float64),
        "beA": np.tile(0.5 * be1 + We1[:128].T @ beta, 2).reshape(128, 1),
        "beB": np.tile(0.5 * be1 + We1[128:].T @ beta, 2).reshape(128, 1),
        "bsA": np.tile(0.5 * bs1 + Ws1[:128].T @ beta, 4).reshape(128, 1),
        "bsB": np.tile(0.5 * bs1 + Ws1[128:].T @ beta, 4).reshape(128, 1),
    }
    wb32 = np.zeros((128, W32_COLS), np.float32)
    for k, v in parts32.items():
        o = W32_OFF[k]
        wb32[:, o:o + v.shape[1]] = v
    shared = {"wb16": wb16, "wb32": f32c(wb32)}

    in_maps = []
    for c in range(NCORES):
        xt = f16(nodes[c * BSH:(c + 1) * BSH].reshape(TOK, D).T)
        in_maps.append({"xt": xt, **shared})

    nc = _get_program()
    res = run_bass_kernel_spmd(nc, in_maps, core_ids=list(range(NCORES)))
    LAST_RESULTS = res

    nf = np.empty((B, N, DM), np.float32)
    adj = np.empty((B, N, N), np.float32)
    stg = np.empty((B, N, N), np.float32)
    mask = (1.0 - np.eye(N, dtype=np.float32))
    rows_adj = np.array([32 * (b // 4) + b % 4 for b in range(BSH)])
    rows_str = rows_adj + 4
    for c in range(NCORES):
        r = res.results[c]
        nf_c = r["nf_raw"].reshape(BSH, N, DM) * gamma + beta
        nf[c * BSH:(c + 1) * BSH] = nf_c
        pr = r["pairs"].astype(np.float32)
        adj[c * BSH:(c + 1) * BSH] = \
            pr[0][rows_adj].reshape(BSH, N, N).transpose(0, 2, 1) * mask
        stg[c * BSH:(c + 1) * BSH] = \
            pr[1][rows_adj].reshape(BSH, N, N).transpose(0, 2, 1) * mask
    return nf, adj, stg
